# revision 1
# baseline (speedup 1.0000x reference)
"""LocalMHA (windowed attention) Trainium2 Bass kernel.

Full inputs -> full outputs. Internally: 8-way data-parallel over
(batch, token-half) shards; each NeuronCore runs the complete block on
4096 tokens (32 windows of 128). No collectives.

Problem (hardcoded):
  x: (4, 1024, 8192) f32, DIM=1024, HEADS=16, DIM_HEAD=64, WINDOW=128
  out = W_out @ attn(LN(x)) + x   (per reference.py)
"""

import numpy as np
import ml_dtypes

import concourse.bass as bass
import concourse.bacc as bacc
import concourse.tile as tile
from concourse import mybir
from concourse.bass_utils import run_bass_kernel_spmd

BF16 = mybir.dt.bfloat16
F32 = mybir.dt.float32

B, DIM, T = 4, 1024, 8192
HEADS, DHEAD, WIN = 16, 64, 128
NCORE = 8
NTOK = (B * T) // NCORE          # 4096 tokens per core
NT = 512                         # token tile
NTILES = NTOK // NT              # 8
KC = DIM // 128                  # 8 contraction chunks
WPT = NT // WIN                  # 4 windows per token tile
EPS = 1e-5

_CACHED = {}
PHASE_LOG = []

# build-time tuning knobs (swept via TimelineSim)
CFG = dict(
    sel_dma=True,       # denom broadcast via DRAM-bounce DMA vs sel matmuls
    sq_gpsimd=False,    # x^2 on gpsimd vs ACT
    psa_bufs=5,
    psb_bufs=1,
)


def _mark(nc, phase):
    PHASE_LOG.append((phase, len(nc.inst_map)))


def _legalize_waits(nc):
    """This toolchain's walrus encodes at most ONE sem-wait per instruction
    (ISA EVENTS struct has a single wait slot) and errors with 'Too many sync
    wait commands' on Tile's multi-wait output. Split: hoist all but one wait
    onto same-engine ENGINE_NOPs inserted immediately before the instruction
    (engine stalls there first -> identical ordering semantics)."""
    eng_map = {
        mybir.EngineType.PE: nc.tensor,
        mybir.EngineType.Activation: nc.scalar,
        mybir.EngineType.DVE: nc.vector,
        mybir.EngineType.Pool: nc.gpsimd,
        mybir.EngineType.SP: nc.sync,
    }
    for f in nc.m.functions:
        for bb in f.blocks:
            lst = bb.instructions  # live list
            need = [
                i for i in lst
                if i.sync_info is not None and len(i.sync_info.on_wait) > 1
            ]
            for inst in need:
                si = inst.sync_info
                waits = list(si.on_wait)
                nops = []
                for w in waits[:-1]:
                    eng = eng_map[inst.engine]
                    bnop = eng.isa(
                        nc.isa.Opcode.NEURON_ISA_TPB_OPCODE_ENGINE_NOP, {}
                    )
                    ni = bnop.ins
                    # engine_nop appended to the current bb; remove it
                    removed = False
                    for f2 in nc.m.functions:
                        for bb2 in f2.blocks:
                            l2 = bb2.instructions
                            if l2 and l2[-1] is ni:
                                l2.pop()
                                removed = True
                                break
                        if removed:
                            break
                    assert removed, "could not relocate wait nop"
                    ni.sync_info = mybir.SyncInfo(on_wait=[w], on_update=[])
                    nops.append(ni)
                inst.sync_info = mybir.SyncInfo(
                    on_wait=[waits[-1]], on_update=list(si.on_update)
                )
                idx = None
                for j in range(len(lst)):
                    if lst[j] is inst:
                        idx = j
                        break
                assert idx is not None
                for k, ni in enumerate(nops):
                    lst.insert(idx + k, ni)
    return nc


def _build_bass(reps=1):
    # Bacc (not plain Bass): its finalize() pipeline runs
    # generate_event_semaphores, which splits Tile's multi-wait sync into the
    # 1-wait-per-instruction form this walrus requires.
    nc = bacc.Bacc("TRN2", target_bir_lowering=False)

    # ---- DRAM I/O ----
    x_d = nc.dram_tensor("x", [DIM, NTOK], F32, kind="ExternalInput")
    # q,k weights, ln_w folded in: (c, m) m in [0, 2048)
    wqk_d = nc.dram_tensor("wqk", [DIM, 2 * DIM], BF16, kind="ExternalInput")
    # v weights: (c, m) m in [0, 1024)
    wv_d = nc.dram_tensor("wv", [DIM, DIM], BF16, kind="ExternalInput")
    # out-proj weights w_out.T: (c, c_out)
    wo_d = nc.dram_tensor("wo", [DIM, DIM], BF16, kind="ExternalInput")
    # rank-1 LN-mean correction rows: a[m] = sum_c W'[c, m]
    aqk_d = nc.dram_tensor("aqk", [1, 2 * DIM], BF16, kind="ExternalInput")
    avbv_d = nc.dram_tensor("avbv", [2, DIM], BF16, kind="ExternalInput")
    # biases (from ln_b): per (partition, chunk) for q,k; row for v
    bqk_d = nc.dram_tensor("bqk", [128, 16], F32, kind="ExternalInput")
    # rope tables, 2 heads stacked (128, 128); f32 copy of sin for psum TT
    cosb_d = nc.dram_tensor("cosb", [128, WIN], BF16, kind="ExternalInput")
    sinf_d = nc.dram_tensor("sinf", [128, WIN], F32, kind="ExternalInput")
    # rotate-half matrix (lhsT), block-diag for 2 heads
    st_d = nc.dram_tensor("st128", [128, 128], BF16, kind="ExternalInput")
    # eye-columns for denominator accumulation: E[:, h, m] = (m == h)
    e16_d = nc.dram_tensor("e16", [128, HEADS, HEADS], BF16, kind="ExternalInput")
    sel_d = nc.dram_tensor("sel", [HEADS, KC, 128], BF16, kind="ExternalInput") if not CFG["sel_dma"] else None
    ones_col_d = nc.dram_tensor("ones_col", [128, 1], BF16, kind="ExternalInput")
    ones_row_d = nc.dram_tensor("ones_row", [1, 128], BF16, kind="ExternalInput")
    out_d = nc.dram_tensor("out", [DIM, NTOK], F32, kind="ExternalOutput")

    x_r = x_d.ap().rearrange("(kc p) n -> p kc n", p=128)
    out_r = out_d.ap().rearrange("(kc p) n -> p kc n", p=128)

    with tile.TileContext(nc) as tc:
        with (
            tc.tile_pool(name="wpool", bufs=1) as wpool,
            tc.tile_pool(name="xpool", bufs=2) as xpool,
            tc.tile_pool(name="spool", bufs=2) as spool,
            tc.tile_pool(name="qkpool", bufs=1) as qkpool,
            tc.tile_pool(name="tpool", bufs=3) as tpool,
            tc.tile_pool(name="vpool", bufs=2) as vpool,
            tc.tile_pool(name="apool", bufs=2) as apool,
            tc.tile_pool(name="ypool", bufs=2) as ypool,
            tc.tile_pool(name="dpool", bufs=2, space="DRAM") as dpool,
            tc.tile_pool(name="psA", bufs=CFG["psa_bufs"], space="PSUM") as psA,
            tc.tile_pool(name="psD", bufs=1, space="PSUM") as psD,
            tc.tile_pool(name="psB", bufs=CFG["psb_bufs"], space="PSUM") as psB,
        ):
            # ---- resident weights/constants ----
            aqk = wpool.tile([1, 2 * DIM], BF16)
            nc.sync.dma_start(out=aqk, in_=aqk_d.ap())
            avbv = wpool.tile([2, DIM], BF16)
            nc.sync.dma_start(out=avbv, in_=avbv_d.ap())
            bqk = wpool.tile([128, 16], F32)
            nc.sync.dma_start(out=bqk, in_=bqk_d.ap())
            cosb = wpool.tile([128, WIN], BF16)
            nc.sync.dma_start(out=cosb, in_=cosb_d.ap())
            sinf = wpool.tile([128, WIN], F32)
            nc.sync.dma_start(out=sinf, in_=sinf_d.ap())
            st128 = wpool.tile([128, 128], BF16)
            nc.sync.dma_start(out=st128, in_=st_d.ap())
            e16 = wpool.tile([128, HEADS, HEADS], BF16)
            nc.sync.dma_start(out=e16, in_=e16_d.ap())
            if not CFG["sel_dma"]:
                sel = wpool.tile([HEADS, KC, 128], BF16)
                nc.sync.dma_start(out=sel, in_=sel_d.ap())
            ones_col = wpool.tile([128, 1], BF16)
            nc.sync.dma_start(out=ones_col, in_=ones_col_d.ap())
            ones_row = wpool.tile([1, 128], BF16)
            nc.sync.dma_start(out=ones_row, in_=ones_row_d.ap())
            eps_t = wpool.tile([1, 1], F32)
            nc.vector.memset(eps_t, EPS)
            zero128 = wpool.tile([128, 1], F32)
            nc.vector.memset(zero128, 0.0)
            wqk = wpool.tile([128, KC, 2 * DIM], BF16)
            nc.sync.dma_start(out=wqk, in_=wqk_d.ap().rearrange("(kc p) m -> p kc m", p=128))
            wv = wpool.tile([128, KC, DIM], BF16)
            nc.sync.dma_start(out=wv, in_=wv_d.ap().rearrange("(kc p) m -> p kc m", p=128))
            wo = wpool.tile([128, KC, DIM], BF16)
            nc.sync.dma_start(out=wo, in_=wo_d.ap().rearrange("(kc p) m -> p kc m", p=128))

            def bcast_win(ap_2d, nwin):
                """(128, WIN) tile -> (128, nwin, WIN) stride-0 repeat."""
                return bass.AP(
                    tensor=ap_2d.tensor,
                    offset=ap_2d.offset,
                    ap=[ap_2d.ap[0], [0, nwin], ap_2d.ap[1]],
                )

            def prologue(it):
                tb = it * NT
                _mark(nc, f'ln_stats_{it}')
                x8 = xpool.tile([128, KC, NT], F32, tag="x8", bufs=1,
                                name=f"x8_{it}")
                nc.sync.dma_start(out=x8, in_=x_r[:, :, tb : tb + NT])
                xb = xpool.tile([128, KC, NT], BF16, tag="xb", bufs=1,
                                name=f"xb_{it}")
                nc.scalar.copy(
                    out=xb.rearrange("p kc n -> p (kc n)"),
                    in_=x8.rearrange("p kc n -> p (kc n)"),
                )
                # LN stats: sum(x), sum(x^2) over channels via PE
                ps_sum = psA.tile([1, NT], F32, tag="mm1", name=f"ps_sum_{it}")
                ps_sq = psA.tile([1, NT], F32, tag="mm1", name=f"ps_sq_{it}")
                for kc in range(KC):
                    x2 = tpool.tile([128, NT], BF16, tag="tmp", name=f"x2_{it}_{kc}")
                    if CFG["sq_gpsimd"]:
                        nc.gpsimd.tensor_mul(out=x2, in0=xb[:, kc, :], in1=xb[:, kc, :])
                    else:
                        nc.scalar.square(out=x2, in_=xb[:, kc, :])
                    nc.tensor.matmul(
                        ps_sum[:, :], ones_col, xb[:, kc, :],
                        start=(kc == 0), stop=(kc == KC - 1),
                    )
                    nc.tensor.matmul(
                        ps_sq[:, :], ones_col, x2,
                        start=(kc == 0), stop=(kc == KC - 1),
                    )
                ex = spool.tile([1, NT], F32, tag="sa", name=f"ex_{it}")
                nc.scalar.mul(out=ex, in_=ps_sum[:, :], mul=1.0 / DIM)
                ex2 = spool.tile([1, NT], F32, tag="sb", name=f"ex2_{it}")
                nc.scalar.mul(out=ex2, in_=ps_sq[:, :], mul=1.0 / DIM)
                negex2 = spool.tile([1, NT], F32, tag="sc", name=f"negex2_{it}")
                nc.vector.scalar_tensor_tensor(
                    out=negex2, in0=ex, scalar=-1.0, in1=ex,
                    op0=mybir.AluOpType.mult, op1=mybir.AluOpType.mult,
                )
                var = spool.tile([1, NT], F32, tag="sa", name=f"var_{it}")
                nc.vector.tensor_add(out=var, in0=ex2, in1=negex2)
                sd = spool.tile([1, NT], F32, tag="sc", name=f"sd_{it}")
                nc.scalar.activation(
                    out=sd, in_=var, func=mybir.ActivationFunctionType.Sqrt,
                    bias=eps_t[:, :], scale=1.0,
                )
                rstd = spool.tile([1, NT], F32, tag="sb", name=f"rstd_{it}")
                nc.vector.reciprocal_approx_fast(out=rstd, in_=sd)
                nmr = spool.tile([1, NT], F32, tag="sc", name=f"nmr_{it}")
                nc.vector.scalar_tensor_tensor(
                    out=nmr, in0=ex, scalar=-1.0, in1=rstd,
                    op0=mybir.AluOpType.mult, op1=mybir.AluOpType.mult,
                )
                rstd_bf = spool.tile([1, NT], BF16, tag="sbf", bufs=4,
                                     name=f"rstd_bf_{it}")
                nc.scalar.copy(out=rstd_bf, in_=rstd)
                # (2, NT): row 0 = -mu*rstd, row 1 = ones; pairs with avbv
                # for a single K=2 rank-2 correction matmul in the vT path.
                nmr1 = spool.tile([2, NT], BF16, tag="sbf", bufs=4,
                                  name=f"nmr1_{it}")
                nc.vector.memset(nmr1, 1.0)
                nc.scalar.copy(out=nmr1[0:1, :], in_=nmr)
                # broadcast rstd to all partitions via a DRAM bounce (keeps
                # PE out of the serial LN-stats chain; stride-0 partition
                # reads are only legal from DRAM)
                rb = spool.tile([128, NT], BF16, tag="rb", name=f"rb_{it}")
                rdram = dpool.tile([1, NT], BF16, tag="rdram", name=f"rdram_{it}")
                nc.sync.dma_start(out=rdram, in_=rstd_bf)
                rstd_bcast_src = bass.AP(
                    tensor=rdram.tensor, offset=rdram.offset,
                    ap=[[0, 128], rdram.ap[1]],
                )
                nc.sync.dma_start(out=rb, in_=rstd_bcast_src)
                # x_s = xb * rstd (mean handled by rank-1 matmul downstream)
                x_s = xpool.tile([128, KC, NT], BF16, tag="xs", name=f"x_s_{it}")
                rb_b = bass.AP(tensor=rb.tensor, offset=rb.offset,
                               ap=[rb.ap[0], [0, KC], rb.ap[1]])
                nc.vector.tensor_mul(out=x_s, in0=xb, in1=rb_b)
                return dict(it=it, tb=tb, x_s=x_s, nmr1=nmr1)

            def qkrope_phase(st):
                it, x_s, nmr1 = st["it"], st["x_s"], st["nmr1"]
                _mark(nc, f'qkrope_{it}')
                qk = qkpool.tile([128, 16, NT], BF16, tag="qk", name=f"qk_{it}")
                for mc in range(16):
                    ps_qk = psA.tile([128, NT], F32, tag="mm1",
                                     name=f"ps_qk_{it}_{mc}")
                    for kc in range(KC):
                        nc.tensor.matmul(
                            ps_qk[:, :],
                            wqk[:, kc, mc * 128 : (mc + 1) * 128],
                            x_s[:, kc, :],
                            start=(kc == 0), stop=False,
                        )
                    nc.tensor.matmul(
                        ps_qk[:, :],
                        aqk[:, mc * 128 : (mc + 1) * 128],
                        nmr1[0:1, :],
                        start=False, stop=True,
                    )
                    nc.scalar.activation(
                        out=qk[:, mc, :], in_=ps_qk[:, :],
                        func=mybir.ActivationFunctionType.Identity,
                        bias=bqk[:, mc : mc + 1], scale=1.0,
                    )
                    # rope: u = rotate_half(qk) via PE
                    ps_u = psA.tile([128, NT], F32, tag="mm1",
                                    name=f"ps_u_{it}_{mc}")
                    nc.tensor.matmul(ps_u[:, :], st128, qk[:, mc, :],
                                     start=True, stop=True)
                    t2 = tpool.tile([128, NT], BF16, tag="tmp",
                                    name=f"t2_{it}_{mc}")
                    nc.vector.tensor_mul(
                        out=t2, in0=ps_u[:, :], in1=bcast_win(sinf, WPT))
                    tmp = tpool.tile([128, NT], BF16, tag="tmp",
                                     name=f"tmp_{it}_{mc}")
                    nc.gpsimd.tensor_mul(
                        out=tmp, in0=qk[:, mc, :], in1=bcast_win(cosb, WPT))
                    nc.vector.tensor_add(out=qk[:, mc, :], in0=tmp, in1=t2)
                st["qk"] = qk

            def vt_phase(st):
                it, x_s, nmr1 = st["it"], st["x_s"], st["nmr1"]
                _mark(nc, f'vt_{it}')
                vt = vpool.tile([128, WPT, DIM], BF16, tag="vt", name=f"vt_{it}")
                for sub in range(WPT):
                    ps_vt = psB.tile([128, DIM], F32, tag="big",
                                     name=f"ps_vt_{it}_{sub}")
                    for nh in range(2):
                        ncol = slice(nh * 512, (nh + 1) * 512)
                        for kc in range(KC):
                            nc.tensor.matmul(
                                ps_vt[:, ncol],
                                x_s[:, kc, sub * 128 : (sub + 1) * 128],
                                wv[:, kc, ncol],
                                start=(kc == 0), stop=False,
                            )
                        nc.tensor.matmul(
                            ps_vt[:, ncol],
                            nmr1[:, sub * 128 : (sub + 1) * 128],
                            avbv[:, ncol],
                            start=False, stop=True,
                        )
                    nc.scalar.copy(out=vt[:, sub, :], in_=ps_vt[:, :])
                st["vt"] = vt

            def attention_phase(st):
                it, qk, vt = st["it"], st["qk"], st["vt"]
                _mark(nc, f'attn_{it}')
                attn_t = apool.tile([128, KC, NT], BF16, tag="attn", bufs=2,
                                    name=f"attn_t_{it}")
                for wl in range(WPT):
                    wslc = slice(wl * WIN, (wl + 1) * WIN)
                    ps_d = psD.tile([HEADS, WIN], F32, tag="dd",
                                    name=f"ps_d_{it}_{wl}")
                    # parity-split so matmuls with different contraction
                    # row-groups (operand base partition 0 vs 64) never share
                    # a PSUM bank (HW faults otherwise).
                    expt = apool.tile([128, 2, 8 * WIN], BF16, tag="expt",
                                      name=f"expt_{it}_{wl}")
                    for hg in range(2):
                        for par in range(2):
                            ps_sc = psA.tile([128, 4 * WIN], F32, tag="mm1",
                                             name=f"ps_sc_{it}_{wl}_{hg}_{par}")
                            po = par * 64
                            for j in range(4):
                                h = hg * 8 + 2 * j + par
                                qh = qk[po : po + 64, h // 2, wslc]
                                kh = qk[po : po + 64, 8 + h // 2, wslc]
                                nc.tensor.matmul(
                                    ps_sc[:, j * WIN : (j + 1) * WIN],
                                    kh, qh, start=True, stop=True,
                                )
                            nc.scalar.activation(
                                out=expt[:, hg, par * 512 : (par + 1) * 512],
                                in_=ps_sc[:, :],
                                func=mybir.ActivationFunctionType.Exp,
                                bias=zero128[:, :], scale=0.125,
                            )

                    def ecol(hh):
                        return (hh % 2) * 512 + (hh // 2) * WIN

                    for hg in range(2):
                        for hh in range(8):
                            h = hg * 8 + hh
                            nc.tensor.matmul(
                                ps_d[:, :], e16[:, h, :],
                                expt[:, hg, ecol(hh) : ecol(hh) + WIN],
                                start=(h == 0), stop=(h == HEADS - 1),
                            )
                    rd = spool.tile([HEADS, WIN], F32, tag="rd",
                                    name=f"rd_{it}_{wl}")
                    nc.vector.reciprocal_approx_fast(out=rd, in_=ps_d[:, :])
                    rd_bf = spool.tile([HEADS, WIN], BF16, tag="rd",
                                       name=f"rd_bf_{it}_{wl}")
                    nc.scalar.copy(out=rd_bf, in_=rd)
                    bcw = apool.tile([128, KC, WIN], BF16, tag="bc",
                                     name=f"bcw_{it}_{wl}")
                    if CFG["sel_dma"]:
                        rd_dram = dpool.tile([HEADS, WIN], BF16, tag="rd_dram",
                                             name=f"rd_dram_{it}_{wl}")
                        nc.sync.dma_start(out=rd_dram, in_=rd_bf)
                        for pb in range(2):
                            src_ap = bass.AP(
                                tensor=rd_dram.tensor,
                                offset=rd_dram.offset + pb * WIN,
                                ap=[[0, 64], [2 * WIN, KC], [1, WIN]],
                            )
                            nc.sync.dma_start(out=bcw[pb * 64 : (pb + 1) * 64], in_=src_ap)
                    else:
                        for hg in range(2):
                            ps_bc = psA.tile([128, 4 * WIN], F32, tag="mm1",
                                             name=f"ps_bc_{it}_{wl}_{hg}")
                            for cc in range(4):
                                c = hg * 4 + cc
                                nc.tensor.matmul(
                                    ps_bc[:, cc * WIN : (cc + 1) * WIN],
                                    sel[:, c, :], rd_bf, start=True, stop=True,
                                )
                            nc.scalar.copy(out=bcw[:, 4 * hg : 4 * hg + 4, :], in_=ps_bc[:, :])
                    for hg in range(2):
                        ps_at = psA.tile([128, 4 * WIN], F32, tag="mm1",
                                         name=f"ps_at_{it}_{wl}_{hg}")
                        for hh in range(8):
                            h = hg * 8 + hh
                            po = (h % 2) * 64
                            c = (h // 2) % 4
                            nc.tensor.matmul(
                                ps_at[po : po + 64, c * WIN : (c + 1) * WIN],
                                vt[:, wl, h * 64 : (h + 1) * 64],
                                expt[:, hg, ecol(hh) : ecol(hh) + WIN],
                                start=True, stop=True,
                                tile_position=(0, po),
                            )
                        attn = apool.tile([128, 4 * WIN], BF16, tag="attnw",
                                          name=f"attn_{it}_{wl}_{hg}")
                        nc.scalar.copy(out=attn, in_=ps_at[:, :])
                        nc.vector.tensor_mul(
                            out=attn_t[:, 4 * hg : 4 * hg + 4, wslc],
                            in0=attn.rearrange("p (c i) -> p c i", c=4),
                            in1=bcw[:, 4 * hg : 4 * hg + 4, :],
                        )
                st["attn_t"] = attn_t

            def outproj_phase(st):
                it, tb, attn_t = st["it"], st["tb"], st["attn_t"]
                _mark(nc, f'outproj_{it}')
                for mc in range(KC):
                    ps_y = psA.tile([128, NT], F32, tag="mm1",
                                    name=f"ps_y_{it}_{mc}")
                    for kc in range(KC):
                        nc.tensor.matmul(
                            ps_y[:, :],
                            wo[:, kc, mc * 128 : (mc + 1) * 128],
                            attn_t[:, kc, :],
                            start=(kc == 0), stop=(kc == KC - 1),
                        )
                    xres = ypool.tile([128, NT], F32, tag="xres", bufs=2,
                                      name=f"xres_{it}_{mc}")
                    nc.sync.dma_start(out=xres, in_=x_r[:, mc, tb : tb + NT])
                    y = ypool.tile([128, NT], F32, tag="y", name=f"y_{it}_{mc}")
                    nc.vector.tensor_add(
                        out=y, in0=ps_y[:, :], in1=xres,
                    )
                    nc.sync.dma_start(
                        out=out_r[:, mc, tb : tb + NT], in_=y,
                    )

            # software pipeline: emit tile i+1's prologue before tile i's
            # attention so the serial LN chain overlaps PE-heavy phases
            its = [t for _ in range(reps) for t in range(NTILES)]
            states = {0: prologue(its[0])}
            for idx, it in enumerate(its):
                st = states.pop(idx)
                qkrope_phase(st)
                vt_phase(st)
                if idx + 1 < len(its):
                    states[idx + 1] = prologue(its[idx + 1])
                attention_phase(st)
                outproj_phase(st)
    nc.finalize()
    return nc


def _host_prep(x, ln_w, ln_b, w_qkv, w_out):
    """Shared (non-x) device inputs, host-precomputed."""
    bf = ml_dtypes.bfloat16
    wqkv_s = (w_qkv * ln_w[None, :]).astype(np.float32)  # (3C, C) scaled
    wT = np.ascontiguousarray(wqkv_s.T)  # (C, 3C)
    b_qkv = (w_qkv @ ln_b).astype(np.float32)  # (3C,)
    a_qkv = wqkv_s.sum(axis=1).astype(np.float32)  # (3C,)

    ins = {}
    ins["wqk"] = np.ascontiguousarray(wT[:, : 2 * DIM]).astype(bf)
    ins["wv"] = np.ascontiguousarray(wT[:, 2 * DIM :]).astype(bf)
    ins["wo"] = np.ascontiguousarray(w_out.T).astype(bf)
    ins["aqk"] = a_qkv[: 2 * DIM].reshape(1, -1).astype(bf)
    ins["avbv"] = np.stack(
        [a_qkv[2 * DIM :], b_qkv[2 * DIM :]]
    ).astype(bf)
    # q,k bias as (partition, chunk): chunk m -> channels m*128..m*128+127
    ins["bqk"] = np.ascontiguousarray(
        b_qkv[: 2 * DIM].reshape(16, 128).T
    ).astype(np.float32)

    inv_freq = 1.0 / 10000 ** (np.arange(0, DHEAD, 2, dtype=np.float32) / DHEAD)
    pos = np.arange(WIN, dtype=np.float32)
    freqs = np.concatenate([np.outer(pos, inv_freq)] * 2, axis=-1)  # (WIN, 64)
    cos_t = np.cos(freqs).T.astype(np.float32)  # (64, WIN)
    sin_t = np.sin(freqs).T.astype(np.float32)
    ins["cosb"] = np.tile(cos_t, (2, 1)).astype(bf)
    ins["sinf"] = np.tile(sin_t, (2, 1)).astype(np.float32)

    S = np.zeros((DHEAD, DHEAD), np.float32)
    S[: DHEAD // 2, DHEAD // 2 :] = -np.eye(DHEAD // 2)
    S[DHEAD // 2 :, : DHEAD // 2] = np.eye(DHEAD // 2)
    ST = S.T
    st128 = np.zeros((128, 128), np.float32)
    st128[:64, :64] = ST
    st128[64:, 64:] = ST
    ins["st128"] = st128.astype(bf)

    e = np.zeros((128, HEADS, HEADS), np.float32)
    for h in range(HEADS):
        e[:, h, h] = 1.0
    ins["e16"] = e.astype(bf)


    ins["ones_col"] = np.ones((128, 1), np.float32).astype(bf)
    ins["ones_row"] = np.ones((1, 128), np.float32).astype(bf)
    return ins


def kernel(x, ln_w, ln_b, w_qkv, w_out, _want_trace=False):
    x = np.asarray(x, dtype=np.float32)
    shared = _host_prep(
        np.asarray(x, np.float32),
        np.asarray(ln_w, np.float32),
        np.asarray(ln_b, np.float32),
        np.asarray(w_qkv, np.float32),
        np.asarray(w_out, np.float32),
    )

    if "nc" not in _CACHED:
        _CACHED["nc"] = _build_bass()
    nc = _CACHED["nc"]

    in_maps = []
    for core in range(NCORE):
        b, half = core // 2, core % 2
        xs = np.ascontiguousarray(x[b, :, half * NTOK : (half + 1) * NTOK])
        m = dict(shared)
        m["x"] = xs
        in_maps.append(m)

    res = run_bass_kernel_spmd(
        nc, in_maps, core_ids=list(range(NCORE)), trace=_want_trace
    )
    out = np.empty((B, DIM, T), np.float32)
    for core in range(NCORE):
        b, half = core // 2, core % 2
        out[b, :, half * NTOK : (half + 1) * NTOK] = res.results[core]["out"]
    if _want_trace:
        _CACHED["last_trace"] = res
    return out



# revision 11
# speedup vs baseline: 1.4644x; 1.4644x over previous
"""LocalMHA (windowed attention) Trainium2 Bass kernel, fp8 DoubleRow version.

Full inputs -> full outputs. 8-way data-parallel over (batch, token-half)
shards; each NeuronCore runs the whole block on 4096 tokens (32 windows
of 128). No collectives.

Problem (hardcoded):
  x: (4, 1024, 8192) f32, DIM=1024, HEADS=16, DIM_HEAD=64, WINDOW=128
  out = W_out @ attn(LN(x)) + x   (per reference.py)

Numerics: QKV / out projections run in fp8e4m3 DoubleRow mode (weights
scaled x16); LN stats from an fp8 copy of x; attention core in bf16.
All ACT ops stay in the natural_log_exp table set (rstd via ln+exp)
to avoid LoadActFuncSet churn.
"""

import numpy as np
import ml_dtypes

import concourse.bass as bass
import concourse.bacc as bacc
import concourse.tile as tile
from concourse import mybir
from concourse.bass_utils import run_bass_kernel_spmd

BF16 = mybir.dt.bfloat16
F32 = mybir.dt.float32
FP8 = mybir.dt.float8e4
DR = mybir.MatmulPerfMode.DoubleRow

B, DIM, T = 4, 1024, 8192
HEADS, DHEAD, WIN = 16, 64, 128
NCORE = 8
NTOK = (B * T) // NCORE          # 4096 tokens per core
NT = 512                         # token tile
NTILES = NTOK // NT              # 8
KC = DIM // 128                  # 8 contraction chunks
KD = KC // 2                     # 4 DoubleRow chunks (K=256 each)
WPT = NT // WIN                  # 4 windows per token tile
WS = 16.0                        # fp8 weight scale
EPS = 1e-5

_CACHED = {}
PHASE_LOG = []


def _mark(nc, phase):
    PHASE_LOG.append((phase, len(nc.inst_map)))


def _legalize_waits(nc):
    """This toolchain's walrus encodes at most ONE sem-wait per instruction
    (ISA EVENTS struct has a single wait slot) and errors with 'Too many sync
    wait commands' on Tile's multi-wait output. Split: hoist all but one wait
    onto same-engine ENGINE_NOPs inserted immediately before the instruction
    (engine stalls there first -> identical ordering semantics)."""
    eng_map = {
        mybir.EngineType.PE: nc.tensor,
        mybir.EngineType.Activation: nc.scalar,
        mybir.EngineType.DVE: nc.vector,
        mybir.EngineType.Pool: nc.gpsimd,
        mybir.EngineType.SP: nc.sync,
    }
    for f in nc.m.functions:
        for bb in f.blocks:
            lst = bb.instructions  # live list
            need = [
                i for i in lst
                if i.sync_info is not None and len(i.sync_info.on_wait) > 1
            ]
            for inst in need:
                si = inst.sync_info
                waits = list(si.on_wait)
                nops = []
                for w in waits[:-1]:
                    eng = eng_map[inst.engine]
                    bnop = eng.isa(
                        nc.isa.Opcode.NEURON_ISA_TPB_OPCODE_ENGINE_NOP, {}
                    )
                    ni = bnop.ins
                    removed = False
                    for f2 in nc.m.functions:
                        for bb2 in f2.blocks:
                            l2 = bb2.instructions
                            if l2 and l2[-1] is ni:
                                l2.pop()
                                removed = True
                                break
                        if removed:
                            break
                    assert removed, "could not relocate wait nop"
                    ni.sync_info = mybir.SyncInfo(on_wait=[w], on_update=[])
                    nops.append(ni)
                inst.sync_info = mybir.SyncInfo(
                    on_wait=[waits[-1]], on_update=list(si.on_update)
                )
                idx = None
                for j in range(len(lst)):
                    if lst[j] is inst:
                        idx = j
                        break
                assert idx is not None
                for k, ni in enumerate(nops):
                    lst.insert(idx + k, ni)
    return nc


def _build_bass():
    nc = bacc.Bacc("TRN2", target_bir_lowering=False)

    # ---- DRAM I/O ----
    x_d = nc.dram_tensor("x", [DIM, NTOK], F32, kind="ExternalInput")
    wqk_d = nc.dram_tensor("wqk", [DIM, 2 * DIM], FP8, kind="ExternalInput")
    wv_d = nc.dram_tensor("wv", [DIM, DIM], FP8, kind="ExternalInput")
    wo_d = nc.dram_tensor("wo", [DIM, DIM], FP8, kind="ExternalInput")
    # corrections: [a_row; b_row] pairs (x WS, fp8)
    cqk_d = nc.dram_tensor("cqk", [1, 2, 2 * DIM], FP8, kind="ExternalInput")
    cv_d = nc.dram_tensor("cv", [1, 2, DIM], FP8, kind="ExternalInput")
    ones2_d = nc.dram_tensor("ones2", [128, 2, 64], FP8, kind="ExternalInput")
    cosb_d = nc.dram_tensor("cosb", [128, WIN], BF16, kind="ExternalInput")
    sinb_d = nc.dram_tensor("sinb", [128, WIN], BF16, kind="ExternalInput")
    st_d = nc.dram_tensor("st128", [128, 128], BF16, kind="ExternalInput")
    e16_d = nc.dram_tensor("e16", [128, HEADS, HEADS], BF16, kind="ExternalInput")
    sel_d = nc.dram_tensor("sel", [HEADS, KC, 128], BF16, kind="ExternalInput")
    out_d = nc.dram_tensor("out", [DIM, NTOK], F32, kind="ExternalOutput")

    x_r = x_d.ap().rearrange("(kc p) n -> p kc n", p=128)
    out_r = out_d.ap().rearrange("(kc p) n -> p kc n", p=128)

    with tile.TileContext(nc) as tc:
        with (
            tc.tile_pool(name="wpool", bufs=1) as wpool,
            tc.tile_pool(name="xpool", bufs=2) as xpool,
            tc.tile_pool(name="spool", bufs=2) as spool,
            tc.tile_pool(name="qkpool", bufs=1) as qkpool,
            tc.tile_pool(name="rpool", bufs=2) as rpool,
            tc.tile_pool(name="tpool", bufs=2) as tpool,
            tc.tile_pool(name="vpool", bufs=2) as vpool,
            tc.tile_pool(name="apool", bufs=2) as apool,
            tc.tile_pool(name="ypool", bufs=2) as ypool,
            tc.tile_pool(name="psA", bufs=5, space="PSUM") as psA,
            tc.tile_pool(name="psU", bufs=2, space="PSUM") as psU,
            tc.tile_pool(name="psD", bufs=1, space="PSUM") as psD,
        ):
            # ---- resident weights/constants ----
            wqk = wpool.tile([128, KC, 2 * DIM], FP8)
            nc.sync.dma_start(out=wqk, in_=wqk_d.ap().rearrange("(kc p) m -> p kc m", p=128))
            wv = wpool.tile([128, KC, DIM], FP8)
            nc.sync.dma_start(out=wv, in_=wv_d.ap().rearrange("(kc p) m -> p kc m", p=128))
            wo = wpool.tile([128, KC, DIM], FP8)
            nc.sync.dma_start(out=wo, in_=wo_d.ap().rearrange("(kc p) m -> p kc m", p=128))
            cqk = wpool.tile([1, 2, 2 * DIM], FP8)
            nc.sync.dma_start(out=cqk, in_=cqk_d.ap())
            cv = wpool.tile([1, 2, DIM], FP8)
            nc.sync.dma_start(out=cv, in_=cv_d.ap())
            ones2_t = wpool.tile([128, 2, 64], FP8)
            nc.sync.dma_start(out=ones2_t, in_=ones2_d.ap())
            ones2 = ones2_t[:, :, 0:1]
            cosb = wpool.tile([128, WIN], BF16)
            nc.sync.dma_start(out=cosb, in_=cosb_d.ap())
            sinb = wpool.tile([128, WIN], BF16)
            nc.sync.dma_start(out=sinb, in_=sinb_d.ap())
            st128 = wpool.tile([128, 128], BF16)
            nc.sync.dma_start(out=st128, in_=st_d.ap())
            e16 = wpool.tile([128, HEADS, HEADS], BF16)
            nc.sync.dma_start(out=e16, in_=e16_d.ap())
            sel = wpool.tile([HEADS, KC, 128], BF16)
            nc.sync.dma_start(out=sel, in_=sel_d.ap())
            eps_t = wpool.tile([1, 1], F32)
            nc.vector.memset(eps_t, EPS)
            zero128 = wpool.tile([128, 1], F32)
            nc.vector.memset(zero128, 0.0)

            def bcast_win(ap_2d, nwin):
                """(128, WIN) tile -> (128, nwin, WIN) stride-0 repeat."""
                return bass.AP(
                    tensor=ap_2d.tensor,
                    offset=ap_2d.offset,
                    ap=[ap_2d.ap[0], [0, nwin], ap_2d.ap[1]],
                )

            def bcast_kc(ap_2d, n):
                return bass.AP(
                    tensor=ap_2d.tensor,
                    offset=ap_2d.offset,
                    ap=[ap_2d.ap[0], [0, n], ap_2d.ap[1]],
                )

            def ps3(ps_tile, c, inner):
                """(128, c*inner) psum tile viewed as (128, c, inner)."""
                return bass.AP(
                    tensor=ps_tile.tensor, offset=ps_tile.offset,
                    ap=[ps_tile.ap[0], [inner, c], [1, inner]],
                )

            # ================= phases =================

            def stats_phase(it):
                tb = it * NT
                _mark(nc, f'stats_{it}')
                x8 = xpool.tile([128, KC, NT], F32, tag="x8", bufs=2,
                                name=f"x8_{it}")
                nc.sync.dma_start(out=x8, in_=x_r[:, :, tb : tb + NT])
                xf8 = xpool.tile([128, KC, NT], FP8, tag="xf8", bufs=1,
                                 name=f"xf8_{it}")
                xsq8 = xpool.tile([128, KC, NT], FP8, tag="xsq8", bufs=1,
                                  name=f"xsq8_{it}")
                ps_sum = psA.tile([1, NT], F32, tag="mm1", name=f"ps_sum_{it}")
                ps_sq = psA.tile([1, NT], F32, tag="mm1", name=f"ps_sq_{it}")
                for j in range(KD):
                    ksl = slice(2 * j, 2 * j + 2)
                    nc.gpsimd.tensor_copy(
                        out=xf8[:, ksl, :].rearrange("p k n -> p (k n)"),
                        in_=x8[:, ksl, :].rearrange("p k n -> p (k n)"),
                    )
                    nc.scalar.square(
                        out=xsq8[:, ksl, :].rearrange("p k n -> p (k n)"),
                        in_=xf8[:, ksl, :].rearrange("p k n -> p (k n)"),
                    )
                    nc.tensor.matmul(
                        ps_sum[:, :], ones2, xf8[:, ksl, :],
                        start=(j == 0), stop=(j == KD - 1), perf_mode=DR,
                    )
                    nc.tensor.matmul(
                        ps_sq[:, :], ones2, xsq8[:, ksl, :],
                        start=(j == 0), stop=(j == KD - 1), perf_mode=DR,
                    )
                ex = spool.tile([1, NT], F32, tag="sa", bufs=2, name=f"ex_{it}")
                nc.scalar.mul(out=ex, in_=ps_sum[:, :], mul=1.0 / DIM)
                ex2 = spool.tile([1, NT], F32, tag="sb", bufs=1, name=f"ex2_{it}")
                nc.scalar.mul(out=ex2, in_=ps_sq[:, :], mul=1.0 / DIM)
                negex2 = spool.tile([1, NT], F32, tag="sc", bufs=1,
                                    name=f"negex2_{it}")
                nc.vector.scalar_tensor_tensor(
                    out=negex2, in0=ex, scalar=-1.0, in1=ex,
                    op0=mybir.AluOpType.mult, op1=mybir.AluOpType.mult,
                )
                var = spool.tile([1, NT], F32, tag="sa", bufs=2, name=f"var_{it}")
                nc.vector.tensor_add(out=var, in0=ex2, in1=negex2)
                # rstd = exp(-0.5 * ln(var + eps)); keeps ACT in one func set
                lnv = spool.tile([1, NT], F32, tag="sc", bufs=1, name=f"lnv_{it}")
                nc.scalar.activation(
                    out=lnv, in_=var, func=mybir.ActivationFunctionType.Ln,
                    bias=eps_t[:, :], scale=1.0,
                )
                rstd = spool.tile([1, NT], F32, tag="sb", bufs=1,
                                  name=f"rstd_{it}")
                nc.scalar.activation(
                    out=rstd, in_=lnv, func=mybir.ActivationFunctionType.Exp,
                    bias=zero128[0:1, :], scale=-0.5,
                )
                rstd_bf = spool.tile([1, NT], BF16, tag="sbf", bufs=2,
                                     name=f"rstd_bf_{it}")
                nc.scalar.copy(out=rstd_bf, in_=rstd)
                nmr = spool.tile([1, NT], F32, tag="sc", bufs=1, name=f"nmr_{it}")
                nc.vector.scalar_tensor_tensor(
                    out=nmr, in0=ex, scalar=-1.0, in1=rstd,
                    op0=mybir.AluOpType.mult, op1=mybir.AluOpType.mult,
                )
                # (1, 2, NT) fp8: row 0 = -mu*rstd, row 1 = ones
                nmr1 = spool.tile([1, 2, NT], FP8, tag="nmr1", bufs=2,
                                  name=f"nmr1_{it}")
                nc.vector.memset(nmr1, 1.0)
                nc.scalar.copy(out=nmr1[0:1, 0, :], in_=nmr)
                # broadcast rstd to all partitions on gpsimd
                rb = spool.tile([128, NT], BF16, tag="rb", bufs=2,
                                name=f"rb_{it}")
                nc.gpsimd.partition_broadcast(rb, rstd_bf, channels=128)
                # x_s = x * rstd -> fp8  (mean folded via rank-1 corr)
                x_s = xpool.tile([128, KC, NT], FP8, tag="xs", bufs=2,
                                 name=f"x_s_{it}")
                nc.vector.tensor_mul(out=x_s, in0=x8, in1=bcast_kc(rb, KC))
                return dict(it=it, tb=tb, x8=x8, x_s=x_s, nmr1=nmr1)

            def qkrope_phase(st):
                """qk projection fused with rope, 2-mc software offset."""
                it, x_s, nmr1 = st["it"], st["x_s"], st["nmr1"]
                _mark(nc, f'qk_{it}')
                qk_e = qkpool.tile([128, 16, NT], BF16, tag="qke", bufs=1,
                                   name=f"qk_e_{it}")
                qs = {}
                roped = rpool.tile([128, 16, NT], BF16, tag="roped", bufs=2,
                                   name=f"roped_{it}")

                def qk_mc(mc):
                    msl = slice(mc * 128, (mc + 1) * 128)
                    ps_qk = psA.tile([128, NT], F32, tag="mm1",
                                     name=f"ps_qk_{it}_{mc}")
                    for j in range(KD):
                        nc.tensor.matmul(
                            ps_qk[:, :],
                            wqk[:, 2 * j : 2 * j + 2, msl],
                            x_s[:, 2 * j : 2 * j + 2, :],
                            start=(j == 0), stop=False, perf_mode=DR,
                        )
                    nc.tensor.matmul(
                        ps_qk[:, :], cqk[:, :, msl], nmr1,
                        start=False, stop=True, perf_mode=DR,
                    )
                    nc.scalar.copy(out=qk_e[:, mc, :], in_=ps_qk[:, :])
                    qs[mc] = tpool.tile([128, NT], BF16, tag="qs", bufs=4,
                                        name=f"qs_{it}_{mc}")
                    nc.vector.tensor_mul(
                        out=qs[mc], in0=qk_e[:, mc, :],
                        in1=bcast_win(sinb, WPT),
                    )

                def rope_mc(mc):
                    ps_u = psU.tile([128, NT], F32, tag="uu",
                                    name=f"ps_u_{it}_{mc}")
                    nc.tensor.matmul(ps_u[:, :], st128, qs.pop(mc),
                                     start=True, stop=True)
                    qc = tpool.tile([128, NT], BF16, tag="qc", bufs=3,
                                    name=f"qc_{it}_{mc}")
                    nc.gpsimd.tensor_mul(
                        out=qc, in0=qk_e[:, mc, :], in1=bcast_win(cosb, WPT))
                    nc.vector.tensor_add(
                        out=roped[:, mc, :], in0=ps_u[:, :], in1=qc)

                for mc in range(16):
                    qk_mc(mc)
                    if mc >= 2:
                        rope_mc(mc - 2)
                rope_mc(14)
                rope_mc(15)
                st["roped"] = roped

            def v_phase(st):
                it, x_s, nmr1 = st["it"], st["x_s"], st["nmr1"]
                _mark(nc, f'v_{it}')
                vt = vpool.tile([128, WPT, DIM], BF16, tag="vt", bufs=2,
                                name=f"vt_{it}")
                for sub in range(WPT):
                    ssl = slice(sub * 128, (sub + 1) * 128)
                    for nh in range(2):
                        ncol = slice(nh * 512, (nh + 1) * 512)
                        ps_vt = psA.tile([128, 512], F32, tag="mm1",
                                         name=f"ps_vt_{it}_{sub}_{nh}")
                        for j in range(KD):
                            nc.tensor.matmul(
                                ps_vt[:, :],
                                x_s[:, 2 * j : 2 * j + 2, ssl],
                                wv[:, 2 * j : 2 * j + 2, ncol],
                                start=(j == 0), stop=False, perf_mode=DR,
                            )
                        nc.tensor.matmul(
                            ps_vt[:, :], nmr1[:, :, ssl], cv[:, :, ncol],
                            start=False, stop=True, perf_mode=DR,
                        )
                        nc.scalar.copy(out=vt[:, sub, ncol], in_=ps_vt[:, :])
                st["vt"] = vt

            def ecol(hh):
                return (hh % 2) * 512 + (hh // 2) * WIN

            def attn_scores(st, wl):
                it, roped = st["it"], st["roped"]
                _mark(nc, f'attn_{it}_{wl}')
                wslc = slice(wl * WIN, (wl + 1) * WIN)
                expt = apool.tile([128, 2, 8 * WIN], BF16, tag="expt", bufs=2,
                                  name=f"expt_{it}_{wl}")
                # parity-split: matmuls with different operand base partition
                # (0 vs 64) never share a PSUM bank.
                for hg in range(2):
                    for par in range(2):
                        ps_sc = psA.tile([128, 4 * WIN], F32, tag="mm1",
                                         name=f"ps_sc_{it}_{wl}_{hg}_{par}")
                        po = par * 64
                        for j in range(4):
                            h = hg * 8 + 2 * j + par
                            qh = roped[po : po + 64, h // 2, wslc]
                            kh = roped[po : po + 64, 8 + h // 2, wslc]
                            nc.tensor.matmul(
                                ps_sc[:, j * WIN : (j + 1) * WIN],
                                kh, qh, start=True, stop=True,
                            )
                        nc.scalar.activation(
                            out=expt[:, hg, par * 512 : (par + 1) * 512],
                            in_=ps_sc[:, :],
                            func=mybir.ActivationFunctionType.Exp,
                            bias=zero128[:, :], scale=0.125 / (WS * WS),
                        )
                st[f"expt_{wl}"] = expt

            def attn_tail(st, wl):
                it, vt = st["it"], st["vt"]
                expt = st.pop(f"expt_{wl}")
                attn_t = st["attn_t"]
                wslc = slice(wl * WIN, (wl + 1) * WIN)
                ps_d = psD.tile([HEADS, WIN], F32, tag="dd",
                                name=f"ps_d_{it}_{wl}")
                for hg in range(2):
                    for hh in range(8):
                        h = hg * 8 + hh
                        nc.tensor.matmul(
                            ps_d[:, :], e16[:, h, :],
                            expt[:, hg, ecol(hh) : ecol(hh) + WIN],
                            start=(h == 0), stop=(h == HEADS - 1),
                        )
                rd = spool.tile([HEADS, WIN], F32, tag="rd", bufs=2,
                                name=f"rd_{it}_{wl}")
                nc.vector.reciprocal_approx_fast(out=rd, in_=ps_d[:, :])
                rd_bf = spool.tile([HEADS, WIN], BF16, tag="rd", bufs=2,
                                   name=f"rd_bf_{it}_{wl}")
                nc.scalar.copy(out=rd_bf, in_=rd)
                # broadcast rd to (128, kc, WIN) layout via sel matmuls
                bcb = apool.tile([128, KC, WIN], BF16, tag="bcb", bufs=2,
                                 name=f"bcb_{it}_{wl}")
                for hg in range(2):
                    ps_bc = psA.tile([128, 4 * WIN], F32, tag="mm1",
                                     name=f"ps_bc_{it}_{wl}_{hg}")
                    for cc in range(4):
                        c = hg * 4 + cc
                        nc.tensor.matmul(
                            ps_bc[:, cc * WIN : (cc + 1) * WIN],
                            sel[:, c, :], rd_bf, start=True, stop=True,
                        )
                    nc.scalar.copy(
                        out=bcb[:, 4 * hg : 4 * hg + 4, :],
                        in_=ps3(ps_bc, 4, WIN),
                    )
                for hg in range(2):
                    ps_at = psA.tile([128, 4 * WIN], F32, tag="mm1",
                                     name=f"ps_at_{it}_{wl}_{hg}")
                    for hh in range(8):
                        h = hg * 8 + hh
                        po = (h % 2) * 64
                        c = (h // 2) % 4
                        nc.tensor.matmul(
                            ps_at[po : po + 64, c * WIN : (c + 1) * WIN],
                            vt[:, wl, h * 64 : (h + 1) * 64],
                            expt[:, hg, ecol(hh) : ecol(hh) + WIN],
                            start=True, stop=True,
                            tile_position=(0, po),
                        )
                    # fused evac + normalize: psum * bcast -> fp8 attn_t
                    nc.vector.tensor_mul(
                        out=attn_t[:, 4 * hg : 4 * hg + 4, wslc],
                        in0=ps3(ps_at, 4, WIN),
                        in1=bcb[:, 4 * hg : 4 * hg + 4, :],
                    )

            def outproj_phase(st):
                it, tb, attn_t, x8 = st["it"], st["tb"], st["attn_t"], st["x8"]
                _mark(nc, f'outproj_{it}')
                for mc in range(KC):
                    msl = slice(mc * 128, (mc + 1) * 128)
                    ps_y = psA.tile([128, NT], F32, tag="mm1",
                                    name=f"ps_y_{it}_{mc}")
                    for j in range(KD):
                        nc.tensor.matmul(
                            ps_y[:, :],
                            wo[:, 2 * j : 2 * j + 2, msl],
                            attn_t[:, 2 * j : 2 * j + 2, :],
                            start=(j == 0), stop=(j == KD - 1), perf_mode=DR,
                        )
                    y = ypool.tile([128, NT], F32, tag="y", name=f"y_{it}_{mc}")
                    nc.vector.scalar_tensor_tensor(
                        out=y, in0=ps_y[:, :], scalar=1.0 / (WS * WS),
                        in1=x8[:, mc, :],
                        op0=mybir.AluOpType.mult, op1=mybir.AluOpType.add,
                    )
                    nc.sync.dma_start(
                        out=out_r[:, mc, tb : tb + NT], in_=y,
                    )

            def attn_all(st):
                attn_scores(st, 0)
                attn_scores(st, 1)
                attn_tail(st, 0)
                attn_scores(st, 2)
                attn_tail(st, 1)
                attn_scores(st, 3)
                attn_tail(st, 2)
                attn_tail(st, 3)

            # ============ software pipeline ============
            # per iter i: stats(i) | attn(i-1) windows pipelined | qk+rope(i)
            #             | outproj(i-1) | v(i)
            prev = None
            for it in range(NTILES):
                st = stats_phase(it)
                st["attn_t"] = apool.tile([128, KC, NT], FP8, tag="attn",
                                          bufs=2, name=f"attn_t_{it}")
                if prev is not None:
                    attn_all(prev)
                qkrope_phase(st)
                if prev is not None:
                    outproj_phase(prev)
                v_phase(st)
                prev = st
            attn_all(prev)
            outproj_phase(prev)

    nc.finalize()
    return nc


def _host_prep(x, ln_w, ln_b, w_qkv, w_out):
    """Shared (non-x) device inputs, host-precomputed."""
    f8 = ml_dtypes.float8_e4m3fn if hasattr(ml_dtypes, 'float8_e4m3fn') \
        else ml_dtypes.float8_e4m3
    bf = ml_dtypes.bfloat16
    wqkv_s = (w_qkv * ln_w[None, :]).astype(np.float32)  # (3C, C) scaled
    wT = np.ascontiguousarray(wqkv_s.T)  # (C, 3C)
    b_qkv = (w_qkv @ ln_b).astype(np.float32)  # (3C,)
    a_qkv = wqkv_s.sum(axis=1).astype(np.float32)  # (3C,)

    ins = {}
    ins["wqk"] = np.ascontiguousarray(wT[:, : 2 * DIM] * WS).astype(f8)
    ins["wv"] = np.ascontiguousarray(wT[:, 2 * DIM :] * WS).astype(f8)
    ins["wo"] = np.ascontiguousarray(w_out.T * WS).astype(f8)
    ins["cqk"] = np.stack(
        [a_qkv[: 2 * DIM] * WS, b_qkv[: 2 * DIM] * WS]
    )[None].astype(f8)
    ins["cv"] = np.stack(
        [a_qkv[2 * DIM :] * WS, b_qkv[2 * DIM :] * WS]
    )[None].astype(f8)
    ins["ones2"] = np.ones((128, 2, 64), np.float32).astype(f8)

    inv_freq = 1.0 / 10000 ** (np.arange(0, DHEAD, 2, dtype=np.float32) / DHEAD)
    pos = np.arange(WIN, dtype=np.float32)
    freqs = np.concatenate([np.outer(pos, inv_freq)] * 2, axis=-1)  # (WIN, 64)
    cos_t = np.cos(freqs).T.astype(np.float32)  # (64, WIN)
    sin_t = np.sin(freqs).T.astype(np.float32)
    ins["cosb"] = np.tile(cos_t, (2, 1)).astype(bf)
    ins["sinb"] = np.tile(sin_t, (2, 1)).astype(bf)

    S = np.zeros((DHEAD, DHEAD), np.float32)
    S[: DHEAD // 2, DHEAD // 2 :] = -np.eye(DHEAD // 2)
    S[DHEAD // 2 :, : DHEAD // 2] = np.eye(DHEAD // 2)
    ST = S.T
    st128 = np.zeros((128, 128), np.float32)
    st128[:64, :64] = ST
    st128[64:, 64:] = ST
    ins["st128"] = st128.astype(bf)

    e = np.zeros((128, HEADS, HEADS), np.float32)
    for h in range(HEADS):
        e[:, h, h] = 1.0
    ins["e16"] = e.astype(bf)

    # sel[h, c, p] = 1 iff head(p, c) == h, i.e. h == 2c + (p >= 64)
    s = np.zeros((HEADS, KC, 128), np.float32)
    for c in range(KC):
        s[2 * c, c, :64] = 1.0
        s[2 * c + 1, c, 64:] = 1.0
    ins["sel"] = s.astype(bf)
    return ins


def kernel(x, ln_w, ln_b, w_qkv, w_out, _want_trace=False):
    x = np.asarray(x, dtype=np.float32)
    shared = _host_prep(
        np.asarray(x, np.float32),
        np.asarray(ln_w, np.float32),
        np.asarray(ln_b, np.float32),
        np.asarray(w_qkv, np.float32),
        np.asarray(w_out, np.float32),
    )

    if "nc" not in _CACHED:
        _CACHED["nc"] = _build_bass()
    nc = _CACHED["nc"]

    in_maps = []
    for core in range(NCORE):
        b, half = core // 2, core % 2
        xs = np.ascontiguousarray(x[b, :, half * NTOK : (half + 1) * NTOK])
        m = dict(shared)
        m["x"] = xs
        in_maps.append(m)

    res = run_bass_kernel_spmd(
        nc, in_maps, core_ids=list(range(NCORE)), trace=_want_trace
    )
    out = np.empty((B, DIM, T), np.float32)
    for core in range(NCORE):
        b, half = core // 2, core % 2
        out[b, :, half * NTOK : (half + 1) * NTOK] = res.results[core]["out"]
    if _want_trace:
        _CACHED["last_trace"] = res
    return out


# revision 12
# speedup vs baseline: 1.5884x; 1.0847x over previous
"""LocalMHA (windowed attention) Trainium2 Bass kernel, fp8 DoubleRow version.

Full inputs -> full outputs. 8-way data-parallel over (batch, token-half)
shards; each NeuronCore runs the whole block on 4096 tokens (32 windows
of 128). No collectives.

Problem (hardcoded):
  x: (4, 1024, 8192) f32, DIM=1024, HEADS=16, DIM_HEAD=64, WINDOW=128
  out = W_out @ attn(LN(x)) + x   (per reference.py)

Numerics: QKV / out projections run in fp8e4m3 DoubleRow mode (weights
scaled x16); LN stats from an fp8 copy of x; attention core in bf16.
All ACT ops stay in the natural_log_exp table set (rstd via ln+exp)
to avoid LoadActFuncSet churn.
"""

import numpy as np
import ml_dtypes

import concourse.bass as bass
import concourse.bacc as bacc
import concourse.tile as tile
from concourse import mybir
from concourse.bass_utils import run_bass_kernel_spmd

BF16 = mybir.dt.bfloat16
F32 = mybir.dt.float32
FP8 = mybir.dt.float8e4
DR = mybir.MatmulPerfMode.DoubleRow

B, DIM, T = 4, 1024, 8192
HEADS, DHEAD, WIN = 16, 64, 128
NCORE = 8
NTOK = (B * T) // NCORE          # 4096 tokens per core
NT = 512                         # token tile
NTILES = NTOK // NT              # 8
KC = DIM // 128                  # 8 contraction chunks
KD = KC // 2                     # 4 DoubleRow chunks (K=256 each)
WPT = NT // WIN                  # 4 windows per token tile
WS = 16.0                        # fp8 weight scale
EPS = 1e-5

_CACHED = {}
PHASE_LOG = []


def _mark(nc, phase):
    PHASE_LOG.append((phase, len(nc.inst_map)))


def _legalize_waits(nc):
    """This toolchain's walrus encodes at most ONE sem-wait per instruction
    (ISA EVENTS struct has a single wait slot) and errors with 'Too many sync
    wait commands' on Tile's multi-wait output. Split: hoist all but one wait
    onto same-engine ENGINE_NOPs inserted immediately before the instruction
    (engine stalls there first -> identical ordering semantics)."""
    eng_map = {
        mybir.EngineType.PE: nc.tensor,
        mybir.EngineType.Activation: nc.scalar,
        mybir.EngineType.DVE: nc.vector,
        mybir.EngineType.Pool: nc.gpsimd,
        mybir.EngineType.SP: nc.sync,
    }
    for f in nc.m.functions:
        for bb in f.blocks:
            lst = bb.instructions  # live list
            need = [
                i for i in lst
                if i.sync_info is not None and len(i.sync_info.on_wait) > 1
            ]
            for inst in need:
                si = inst.sync_info
                waits = list(si.on_wait)
                nops = []
                for w in waits[:-1]:
                    eng = eng_map[inst.engine]
                    bnop = eng.isa(
                        nc.isa.Opcode.NEURON_ISA_TPB_OPCODE_ENGINE_NOP, {}
                    )
                    ni = bnop.ins
                    removed = False
                    for f2 in nc.m.functions:
                        for bb2 in f2.blocks:
                            l2 = bb2.instructions
                            if l2 and l2[-1] is ni:
                                l2.pop()
                                removed = True
                                break
                        if removed:
                            break
                    assert removed, "could not relocate wait nop"
                    ni.sync_info = mybir.SyncInfo(on_wait=[w], on_update=[])
                    nops.append(ni)
                inst.sync_info = mybir.SyncInfo(
                    on_wait=[waits[-1]], on_update=list(si.on_update)
                )
                idx = None
                for j in range(len(lst)):
                    if lst[j] is inst:
                        idx = j
                        break
                assert idx is not None
                for k, ni in enumerate(nops):
                    lst.insert(idx + k, ni)
    return nc


def _build_bass():
    nc = bacc.Bacc("TRN2", target_bir_lowering=False)

    # ---- DRAM I/O ----
    x_d = nc.dram_tensor("x", [DIM, NTOK], F32, kind="ExternalInput")
    wqk_d = nc.dram_tensor("wqk", [DIM, 2 * DIM], FP8, kind="ExternalInput")
    wv_d = nc.dram_tensor("wv", [DIM, DIM], FP8, kind="ExternalInput")
    wo_d = nc.dram_tensor("wo", [DIM, DIM], FP8, kind="ExternalInput")
    # corrections: [a_row; b_row] pairs (x WS, fp8)
    cqk_d = nc.dram_tensor("cqk", [1, 2, 2 * DIM], FP8, kind="ExternalInput")
    cv_d = nc.dram_tensor("cv", [1, 2, DIM], FP8, kind="ExternalInput")
    ones2_d = nc.dram_tensor("ones2", [128, 2, 64], FP8, kind="ExternalInput")
    cosb_d = nc.dram_tensor("cosb", [128, WIN], BF16, kind="ExternalInput")
    sinb_d = nc.dram_tensor("sinb", [128, WIN], BF16, kind="ExternalInput")
    st_d = nc.dram_tensor("st128", [128, 128], BF16, kind="ExternalInput")
    e16_d = nc.dram_tensor("e16", [128, HEADS, HEADS], BF16, kind="ExternalInput")
    sel_d = nc.dram_tensor("sel", [HEADS, KC, 128], BF16, kind="ExternalInput")
    out_d = nc.dram_tensor("out", [DIM, NTOK], F32, kind="ExternalOutput")

    x_r = x_d.ap().rearrange("(kc p) n -> p kc n", p=128)
    out_r = out_d.ap().rearrange("(kc p) n -> p kc n", p=128)

    with tile.TileContext(nc) as tc:
        with (
            tc.tile_pool(name="wpool", bufs=1) as wpool,
            tc.tile_pool(name="xpool", bufs=2) as xpool,
            tc.tile_pool(name="spool", bufs=2) as spool,
            tc.tile_pool(name="qkpool", bufs=1) as qkpool,
            tc.tile_pool(name="rpool", bufs=2) as rpool,
            tc.tile_pool(name="tpool", bufs=2) as tpool,
            tc.tile_pool(name="vpool", bufs=2) as vpool,
            tc.tile_pool(name="apool", bufs=2) as apool,
            tc.tile_pool(name="ypool", bufs=2) as ypool,
            tc.tile_pool(name="psA", bufs=5, space="PSUM") as psA,
            tc.tile_pool(name="psU", bufs=2, space="PSUM") as psU,
            tc.tile_pool(name="psD", bufs=1, space="PSUM") as psD,
        ):
            # ---- resident weights/constants ----
            wqk = wpool.tile([128, KC, 2 * DIM], FP8)
            nc.sync.dma_start(out=wqk, in_=wqk_d.ap().rearrange("(kc p) m -> p kc m", p=128))
            wv = wpool.tile([128, KC, DIM], FP8)
            nc.sync.dma_start(out=wv, in_=wv_d.ap().rearrange("(kc p) m -> p kc m", p=128))
            wo = wpool.tile([128, KC, DIM], FP8)
            nc.sync.dma_start(out=wo, in_=wo_d.ap().rearrange("(kc p) m -> p kc m", p=128))
            cqk = wpool.tile([1, 2, 2 * DIM], FP8)
            nc.sync.dma_start(out=cqk, in_=cqk_d.ap())
            cv = wpool.tile([1, 2, DIM], FP8)
            nc.sync.dma_start(out=cv, in_=cv_d.ap())
            ones2_t = wpool.tile([128, 2, 64], FP8)
            nc.sync.dma_start(out=ones2_t, in_=ones2_d.ap())
            ones2 = ones2_t[:, :, 0:1]
            cosb = wpool.tile([128, WIN], BF16)
            nc.sync.dma_start(out=cosb, in_=cosb_d.ap())
            sinb = wpool.tile([128, WIN], BF16)
            nc.sync.dma_start(out=sinb, in_=sinb_d.ap())
            st128 = wpool.tile([128, 128], BF16)
            nc.sync.dma_start(out=st128, in_=st_d.ap())
            e16 = wpool.tile([128, HEADS, HEADS], BF16)
            nc.sync.dma_start(out=e16, in_=e16_d.ap())
            sel = wpool.tile([HEADS, KC, 128], BF16)
            nc.sync.dma_start(out=sel, in_=sel_d.ap())
            eps_t = wpool.tile([1, 1], F32)
            nc.vector.memset(eps_t, EPS)
            zero128 = wpool.tile([128, 1], F32)
            nc.vector.memset(zero128, 0.0)

            def bcast_win(ap_2d, nwin):
                """(128, WIN) tile -> (128, nwin, WIN) stride-0 repeat."""
                return bass.AP(
                    tensor=ap_2d.tensor,
                    offset=ap_2d.offset,
                    ap=[ap_2d.ap[0], [0, nwin], ap_2d.ap[1]],
                )

            def bcast_kc(ap_2d, n):
                return bass.AP(
                    tensor=ap_2d.tensor,
                    offset=ap_2d.offset,
                    ap=[ap_2d.ap[0], [0, n], ap_2d.ap[1]],
                )

            def ps3(ps_tile, c, inner):
                """(128, c*inner) psum tile viewed as (128, c, inner)."""
                return bass.AP(
                    tensor=ps_tile.tensor, offset=ps_tile.offset,
                    ap=[ps_tile.ap[0], [inner, c], [1, inner]],
                )

            # ================= phases =================

            def stats_phase(it):
                tb = it * NT
                _mark(nc, f'stats_{it}')
                x8 = xpool.tile([128, KC, NT], F32, tag="x8", bufs=1,
                                name=f"x8_{it}")
                nc.sync.dma_start(out=x8, in_=x_r[:, :, tb : tb + NT])
                xf8 = xpool.tile([128, KC, NT], FP8, tag="xf8", bufs=1,
                                 name=f"xf8_{it}")
                xsq8 = xpool.tile([128, KC, NT], FP8, tag="xsq8", bufs=1,
                                  name=f"xsq8_{it}")
                ps_sum = psA.tile([1, NT], F32, tag="mm1", name=f"ps_sum_{it}")
                ps_sq = psA.tile([1, NT], F32, tag="mm1", name=f"ps_sq_{it}")
                for j in range(KD):
                    ksl = slice(2 * j, 2 * j + 2)
                    nc.gpsimd.tensor_copy(
                        out=xf8[:, ksl, :].rearrange("p k n -> p (k n)"),
                        in_=x8[:, ksl, :].rearrange("p k n -> p (k n)"),
                    )
                    if j % 2 == 0:
                        nc.scalar.square(
                            out=xsq8[:, ksl, :].rearrange("p k n -> p (k n)"),
                            in_=xf8[:, ksl, :].rearrange("p k n -> p (k n)"),
                        )
                    else:
                        nc.gpsimd.tensor_mul(
                            out=xsq8[:, ksl, :].rearrange("p k n -> p (k n)"),
                            in0=xf8[:, ksl, :].rearrange("p k n -> p (k n)"),
                            in1=xf8[:, ksl, :].rearrange("p k n -> p (k n)"),
                        )
                    nc.tensor.matmul(
                        ps_sum[:, :], ones2, xf8[:, ksl, :],
                        start=(j == 0), stop=(j == KD - 1), perf_mode=DR,
                    )
                    nc.tensor.matmul(
                        ps_sq[:, :], ones2, xsq8[:, ksl, :],
                        start=(j == 0), stop=(j == KD - 1), perf_mode=DR,
                    )
                ex = spool.tile([1, NT], F32, tag="sa", bufs=2, name=f"ex_{it}")
                nc.scalar.mul(out=ex, in_=ps_sum[:, :], mul=1.0 / DIM)
                ex2 = spool.tile([1, NT], F32, tag="sb", bufs=1, name=f"ex2_{it}")
                nc.scalar.mul(out=ex2, in_=ps_sq[:, :], mul=1.0 / DIM)
                negex2 = spool.tile([1, NT], F32, tag="sc", bufs=1,
                                    name=f"negex2_{it}")
                nc.vector.scalar_tensor_tensor(
                    out=negex2, in0=ex, scalar=-1.0, in1=ex,
                    op0=mybir.AluOpType.mult, op1=mybir.AluOpType.mult,
                )
                var = spool.tile([1, NT], F32, tag="sa", bufs=2, name=f"var_{it}")
                nc.vector.tensor_add(out=var, in0=ex2, in1=negex2)
                # rstd = exp(-0.5 * ln(var + eps)); keeps ACT in one func set
                lnv = spool.tile([1, NT], F32, tag="sc", bufs=1, name=f"lnv_{it}")
                nc.scalar.activation(
                    out=lnv, in_=var, func=mybir.ActivationFunctionType.Ln,
                    bias=eps_t[:, :], scale=1.0,
                )
                rstd = spool.tile([1, NT], F32, tag="sb", bufs=1,
                                  name=f"rstd_{it}")
                nc.scalar.activation(
                    out=rstd, in_=lnv, func=mybir.ActivationFunctionType.Exp,
                    bias=zero128[0:1, :], scale=-0.5,
                )
                rstd_bf = spool.tile([1, NT], BF16, tag="sbf", bufs=1,
                                     name=f"rstd_bf_{it}")
                nc.gpsimd.tensor_copy(out=rstd_bf, in_=rstd)
                nmr = spool.tile([1, NT], F32, tag="sc", bufs=1, name=f"nmr_{it}")
                nc.vector.scalar_tensor_tensor(
                    out=nmr, in0=ex, scalar=-1.0, in1=rstd,
                    op0=mybir.AluOpType.mult, op1=mybir.AluOpType.mult,
                )
                # (1, 2, NT) fp8: row 0 = -mu*rstd, row 1 = ones
                nmr1 = spool.tile([1, 2, NT], FP8, tag="nmr1", bufs=1,
                                  name=f"nmr1_{it}")
                nc.vector.memset(nmr1, 1.0)
                nc.gpsimd.tensor_copy(out=nmr1[0:1, 0, :], in_=nmr)
                # broadcast rstd to all partitions on gpsimd
                rb = spool.tile([128, NT], BF16, tag="rb", bufs=1,
                                name=f"rb_{it}")
                nc.gpsimd.partition_broadcast(rb, rstd_bf, channels=128)
                # x_s = x * rstd -> fp8  (mean folded via rank-1 corr)
                x_s = xpool.tile([128, KC, NT], FP8, tag="xs", bufs=2,
                                 name=f"x_s_{it}")
                nc.vector.tensor_mul(out=x_s, in0=x8, in1=bcast_kc(rb, KC))
                return dict(it=it, tb=tb, x_s=x_s, nmr1=nmr1)

            def qkrope_phase(st):
                """qk projection fused with rope, 2-mc software offset."""
                it, x_s, nmr1 = st["it"], st["x_s"], st["nmr1"]
                _mark(nc, f'qk_{it}')
                qk_e = qkpool.tile([128, 16, NT], BF16, tag="qke", bufs=1,
                                   name=f"qk_e_{it}")
                qs = {}
                roped = rpool.tile([128, 16, NT], BF16, tag="roped", bufs=2,
                                   name=f"roped_{it}")

                def qk_mc(mc):
                    msl = slice(mc * 128, (mc + 1) * 128)
                    ps_qk = psA.tile([128, NT], F32, tag="mm1",
                                     name=f"ps_qk_{it}_{mc}")
                    for j in range(KD):
                        nc.tensor.matmul(
                            ps_qk[:, :],
                            wqk[:, 2 * j : 2 * j + 2, msl],
                            x_s[:, 2 * j : 2 * j + 2, :],
                            start=(j == 0), stop=False, perf_mode=DR,
                        )
                    nc.tensor.matmul(
                        ps_qk[:, :], cqk[:, :, msl], nmr1,
                        start=False, stop=True, perf_mode=DR,
                    )
                    nc.scalar.copy(out=qk_e[:, mc, :], in_=ps_qk[:, :])
                    qs[mc] = tpool.tile([128, NT], BF16, tag="qs", bufs=4,
                                        name=f"qs_{it}_{mc}")
                    nc.vector.tensor_mul(
                        out=qs[mc], in0=qk_e[:, mc, :],
                        in1=bcast_win(sinb, WPT),
                    )

                def rope_mc(mc):
                    ps_u = psU.tile([128, NT], F32, tag="uu",
                                    name=f"ps_u_{it}_{mc}")
                    nc.tensor.matmul(ps_u[:, :], st128, qs.pop(mc),
                                     start=True, stop=True)
                    qc = tpool.tile([128, NT], BF16, tag="qc", bufs=2,
                                    name=f"qc_{it}_{mc}")
                    nc.gpsimd.tensor_mul(
                        out=qc, in0=qk_e[:, mc, :], in1=bcast_win(cosb, WPT))
                    nc.vector.tensor_add(
                        out=roped[:, mc, :], in0=ps_u[:, :], in1=qc)

                for mc in range(16):
                    qk_mc(mc)
                    if mc >= 2:
                        rope_mc(mc - 2)
                rope_mc(14)
                rope_mc(15)
                st["roped"] = roped

            def v_phase(st):
                it, x_s, nmr1 = st["it"], st["x_s"], st["nmr1"]
                _mark(nc, f'v_{it}')
                vt = vpool.tile([128, WPT, DIM], BF16, tag="vt", bufs=2,
                                name=f"vt_{it}")
                for sub in range(WPT):
                    ssl = slice(sub * 128, (sub + 1) * 128)
                    for nh in range(2):
                        ncol = slice(nh * 512, (nh + 1) * 512)
                        ps_vt = psA.tile([128, 512], F32, tag="mm1",
                                         name=f"ps_vt_{it}_{sub}_{nh}")
                        for j in range(KD):
                            nc.tensor.matmul(
                                ps_vt[:, :],
                                x_s[:, 2 * j : 2 * j + 2, ssl],
                                wv[:, 2 * j : 2 * j + 2, ncol],
                                start=(j == 0), stop=False, perf_mode=DR,
                            )
                        nc.tensor.matmul(
                            ps_vt[:, :], nmr1[:, :, ssl], cv[:, :, ncol],
                            start=False, stop=True, perf_mode=DR,
                        )
                        nc.scalar.copy(out=vt[:, sub, ncol], in_=ps_vt[:, :])
                st["vt"] = vt

            def ecol(hh):
                return (hh % 2) * 512 + (hh // 2) * WIN

            def attn_scores(st, wl):
                it, roped = st["it"], st["roped"]
                _mark(nc, f'attn_{it}_{wl}')
                wslc = slice(wl * WIN, (wl + 1) * WIN)
                expt = apool.tile([128, 2, 8 * WIN], BF16, tag="expt", bufs=2,
                                  name=f"expt_{it}_{wl}")
                # parity-split: matmuls with different operand base partition
                # (0 vs 64) never share a PSUM bank.
                for hg in range(2):
                    for par in range(2):
                        ps_sc = psA.tile([128, 4 * WIN], F32, tag="mm1",
                                         name=f"ps_sc_{it}_{wl}_{hg}_{par}")
                        po = par * 64
                        for j in range(4):
                            h = hg * 8 + 2 * j + par
                            qh = roped[po : po + 64, h // 2, wslc]
                            kh = roped[po : po + 64, 8 + h // 2, wslc]
                            nc.tensor.matmul(
                                ps_sc[:, j * WIN : (j + 1) * WIN],
                                kh, qh, start=True, stop=True,
                            )
                        nc.scalar.activation(
                            out=expt[:, hg, par * 512 : (par + 1) * 512],
                            in_=ps_sc[:, :],
                            func=mybir.ActivationFunctionType.Exp,
                            bias=zero128[:, :], scale=0.125 / (WS * WS),
                        )
                st[f"expt_{wl}"] = expt

            def attn_tail(st, wl):
                it, vt = st["it"], st["vt"]
                expt = st.pop(f"expt_{wl}")
                attn_t = st["attn_t"]
                wslc = slice(wl * WIN, (wl + 1) * WIN)
                ps_d = psD.tile([HEADS, WIN], F32, tag="dd",
                                name=f"ps_d_{it}_{wl}")
                for hg in range(2):
                    for hh in range(8):
                        h = hg * 8 + hh
                        nc.tensor.matmul(
                            ps_d[:, :], e16[:, h, :],
                            expt[:, hg, ecol(hh) : ecol(hh) + WIN],
                            start=(h == 0), stop=(h == HEADS - 1),
                        )
                rd = spool.tile([HEADS, WIN], F32, tag="rd", bufs=2,
                                name=f"rd_{it}_{wl}")
                nc.vector.reciprocal_approx_fast(out=rd, in_=ps_d[:, :])
                rd_bf = spool.tile([HEADS, WIN], BF16, tag="rd", bufs=2,
                                   name=f"rd_bf_{it}_{wl}")
                nc.gpsimd.tensor_copy(out=rd_bf, in_=rd)
                # broadcast rd to (128, kc, WIN) layout via sel matmuls
                bcb = apool.tile([128, KC, WIN], BF16, tag="bcb", bufs=2,
                                 name=f"bcb_{it}_{wl}")
                for hg in range(2):
                    ps_bc = psA.tile([128, 4 * WIN], F32, tag="mm1",
                                     name=f"ps_bc_{it}_{wl}_{hg}")
                    for cc in range(4):
                        c = hg * 4 + cc
                        nc.tensor.matmul(
                            ps_bc[:, cc * WIN : (cc + 1) * WIN],
                            sel[:, c, :], rd_bf, start=True, stop=True,
                        )
                    nc.scalar.copy(
                        out=bcb[:, 4 * hg : 4 * hg + 4, :],
                        in_=ps3(ps_bc, 4, WIN),
                    )
                for hg in range(2):
                    ps_at = psA.tile([128, 4 * WIN], F32, tag="mm1",
                                     name=f"ps_at_{it}_{wl}_{hg}")
                    for hh in range(8):
                        h = hg * 8 + hh
                        po = (h % 2) * 64
                        c = (h // 2) % 4
                        nc.tensor.matmul(
                            ps_at[po : po + 64, c * WIN : (c + 1) * WIN],
                            vt[:, wl, h * 64 : (h + 1) * 64],
                            expt[:, hg, ecol(hh) : ecol(hh) + WIN],
                            start=True, stop=True,
                            tile_position=(0, po),
                        )
                    # fused evac + normalize: psum * bcast -> fp8 attn_t
                    nc.vector.tensor_mul(
                        out=attn_t[:, 4 * hg : 4 * hg + 4, wslc],
                        in0=ps3(ps_at, 4, WIN),
                        in1=bcb[:, 4 * hg : 4 * hg + 4, :],
                    )

            def outproj_phase(st):
                it, tb, attn_t = st["it"], st["tb"], st["attn_t"]
                _mark(nc, f'outproj_{it}')
                for mc in range(KC):
                    msl = slice(mc * 128, (mc + 1) * 128)
                    ps_y = psA.tile([128, NT], F32, tag="mm1",
                                    name=f"ps_y_{it}_{mc}")
                    for j in range(KD):
                        nc.tensor.matmul(
                            ps_y[:, :],
                            wo[:, 2 * j : 2 * j + 2, msl],
                            attn_t[:, 2 * j : 2 * j + 2, :],
                            start=(j == 0), stop=(j == KD - 1), perf_mode=DR,
                        )
                    xres = ypool.tile([128, NT], F32, tag="xres", bufs=2,
                                      name=f"xres_{it}_{mc}")
                    nc.sync.dma_start(out=xres, in_=x_r[:, mc, tb : tb + NT])
                    y = ypool.tile([128, NT], F32, tag="y", bufs=4,
                                   name=f"y_{it}_{mc}")
                    nc.vector.scalar_tensor_tensor(
                        out=y, in0=ps_y[:, :], scalar=1.0 / (WS * WS),
                        in1=xres,
                        op0=mybir.AluOpType.mult, op1=mybir.AluOpType.add,
                    )
                    nc.sync.dma_start(
                        out=out_r[:, mc, tb : tb + NT], in_=y,
                    )

            def attn_all(st):
                attn_scores(st, 0)
                attn_scores(st, 1)
                attn_tail(st, 0)
                attn_scores(st, 2)
                attn_tail(st, 1)
                attn_scores(st, 3)
                attn_tail(st, 2)
                attn_tail(st, 3)

            # ============ software pipeline ============
            # per iter i: stats(i) | attn(i-1) windows pipelined | qk+rope(i)
            #             | outproj(i-1) | v(i)
            prev = None
            cur = stats_phase(0)
            for it in range(NTILES):
                cur["attn_t"] = apool.tile([128, KC, NT], FP8, tag="attn",
                                           bufs=2, name=f"attn_t_{it}")
                if prev is not None:
                    attn_all(prev)
                qkrope_phase(cur)
                nxt = stats_phase(it + 1) if it + 1 < NTILES else None
                if prev is not None:
                    outproj_phase(prev)
                v_phase(cur)
                prev, cur = cur, nxt
            attn_all(prev)
            outproj_phase(prev)

    nc.finalize()
    return nc


def _host_prep(x, ln_w, ln_b, w_qkv, w_out):
    """Shared (non-x) device inputs, host-precomputed."""
    f8 = ml_dtypes.float8_e4m3fn if hasattr(ml_dtypes, 'float8_e4m3fn') \
        else ml_dtypes.float8_e4m3
    bf = ml_dtypes.bfloat16
    wqkv_s = (w_qkv * ln_w[None, :]).astype(np.float32)  # (3C, C) scaled
    wT = np.ascontiguousarray(wqkv_s.T)  # (C, 3C)
    b_qkv = (w_qkv @ ln_b).astype(np.float32)  # (3C,)
    a_qkv = wqkv_s.sum(axis=1).astype(np.float32)  # (3C,)

    ins = {}
    ins["wqk"] = np.ascontiguousarray(wT[:, : 2 * DIM] * WS).astype(f8)
    ins["wv"] = np.ascontiguousarray(wT[:, 2 * DIM :] * WS).astype(f8)
    ins["wo"] = np.ascontiguousarray(w_out.T * WS).astype(f8)
    ins["cqk"] = np.stack(
        [a_qkv[: 2 * DIM] * WS, b_qkv[: 2 * DIM] * WS]
    )[None].astype(f8)
    ins["cv"] = np.stack(
        [a_qkv[2 * DIM :] * WS, b_qkv[2 * DIM :] * WS]
    )[None].astype(f8)
    ins["ones2"] = np.ones((128, 2, 64), np.float32).astype(f8)

    inv_freq = 1.0 / 10000 ** (np.arange(0, DHEAD, 2, dtype=np.float32) / DHEAD)
    pos = np.arange(WIN, dtype=np.float32)
    freqs = np.concatenate([np.outer(pos, inv_freq)] * 2, axis=-1)  # (WIN, 64)
    cos_t = np.cos(freqs).T.astype(np.float32)  # (64, WIN)
    sin_t = np.sin(freqs).T.astype(np.float32)
    ins["cosb"] = np.tile(cos_t, (2, 1)).astype(bf)
    ins["sinb"] = np.tile(sin_t, (2, 1)).astype(bf)

    S = np.zeros((DHEAD, DHEAD), np.float32)
    S[: DHEAD // 2, DHEAD // 2 :] = -np.eye(DHEAD // 2)
    S[DHEAD // 2 :, : DHEAD // 2] = np.eye(DHEAD // 2)
    ST = S.T
    st128 = np.zeros((128, 128), np.float32)
    st128[:64, :64] = ST
    st128[64:, 64:] = ST
    ins["st128"] = st128.astype(bf)

    e = np.zeros((128, HEADS, HEADS), np.float32)
    for h in range(HEADS):
        e[:, h, h] = 1.0
    ins["e16"] = e.astype(bf)

    # sel[h, c, p] = 1 iff head(p, c) == h, i.e. h == 2c + (p >= 64)
    s = np.zeros((HEADS, KC, 128), np.float32)
    for c in range(KC):
        s[2 * c, c, :64] = 1.0
        s[2 * c + 1, c, 64:] = 1.0
    ins["sel"] = s.astype(bf)
    return ins


def kernel(x, ln_w, ln_b, w_qkv, w_out, _want_trace=False):
    x = np.asarray(x, dtype=np.float32)
    shared = _host_prep(
        np.asarray(x, np.float32),
        np.asarray(ln_w, np.float32),
        np.asarray(ln_b, np.float32),
        np.asarray(w_qkv, np.float32),
        np.asarray(w_out, np.float32),
    )

    if "nc" not in _CACHED:
        _CACHED["nc"] = _build_bass()
    nc = _CACHED["nc"]

    in_maps = []
    for core in range(NCORE):
        b, half = core // 2, core % 2
        xs = np.ascontiguousarray(x[b, :, half * NTOK : (half + 1) * NTOK])
        m = dict(shared)
        m["x"] = xs
        in_maps.append(m)

    res = run_bass_kernel_spmd(
        nc, in_maps, core_ids=list(range(NCORE)), trace=_want_trace
    )
    out = np.empty((B, DIM, T), np.float32)
    for core in range(NCORE):
        b, half = core // 2, core % 2
        out[b, :, half * NTOK : (half + 1) * NTOK] = res.results[core]["out"]
    if _want_trace:
        _CACHED["last_trace"] = res
    return out


# revision 14
# speedup vs baseline: 1.6361x; 1.0300x over previous
"""LocalMHA (windowed attention) Trainium2 Bass kernel, fp8 DoubleRow version.

Full inputs -> full outputs. 8-way data-parallel over (batch, token-half)
shards; each NeuronCore runs the whole block on 4096 tokens (32 windows
of 128). No collectives.

Problem (hardcoded):
  x: (4, 1024, 8192) f32, DIM=1024, HEADS=16, DIM_HEAD=64, WINDOW=128
  out = W_out @ attn(LN(x)) + x   (per reference.py)

Numerics: QKV / out projections run in fp8e4m3 DoubleRow mode (weights
scaled x16); LN stats from an fp8 copy of x; attention core in bf16.
All ACT ops stay in the natural_log_exp table set (rstd via ln+exp)
to avoid LoadActFuncSet churn.
"""

import numpy as np
import ml_dtypes

import concourse.bass as bass
import concourse.bacc as bacc
import concourse.tile as tile
from concourse import mybir
from concourse.bass_utils import run_bass_kernel_spmd

BF16 = mybir.dt.bfloat16
F32 = mybir.dt.float32
FP8 = mybir.dt.float8e4
DR = mybir.MatmulPerfMode.DoubleRow

B, DIM, T = 4, 1024, 8192
HEADS, DHEAD, WIN = 16, 64, 128
NCORE = 8
NTOK = (B * T) // NCORE          # 4096 tokens per core
NT = 512                         # token tile
NTILES = NTOK // NT              # 8
KC = DIM // 128                  # 8 contraction chunks
KD = KC // 2                     # 4 DoubleRow chunks (K=256 each)
WPT = NT // WIN                  # 4 windows per token tile
WS = 16.0                        # fp8 weight scale
EPS = 1e-5

_CACHED = {}
PHASE_LOG = []


def _mark(nc, phase):
    PHASE_LOG.append((phase, len(nc.inst_map)))


def _legalize_waits(nc):
    """This toolchain's walrus encodes at most ONE sem-wait per instruction
    (ISA EVENTS struct has a single wait slot) and errors with 'Too many sync
    wait commands' on Tile's multi-wait output. Split: hoist all but one wait
    onto same-engine ENGINE_NOPs inserted immediately before the instruction
    (engine stalls there first -> identical ordering semantics)."""
    eng_map = {
        mybir.EngineType.PE: nc.tensor,
        mybir.EngineType.Activation: nc.scalar,
        mybir.EngineType.DVE: nc.vector,
        mybir.EngineType.Pool: nc.gpsimd,
        mybir.EngineType.SP: nc.sync,
    }
    for f in nc.m.functions:
        for bb in f.blocks:
            lst = bb.instructions  # live list
            need = [
                i for i in lst
                if i.sync_info is not None and len(i.sync_info.on_wait) > 1
            ]
            for inst in need:
                si = inst.sync_info
                waits = list(si.on_wait)
                nops = []
                for w in waits[:-1]:
                    eng = eng_map[inst.engine]
                    bnop = eng.isa(
                        nc.isa.Opcode.NEURON_ISA_TPB_OPCODE_ENGINE_NOP, {}
                    )
                    ni = bnop.ins
                    removed = False
                    for f2 in nc.m.functions:
                        for bb2 in f2.blocks:
                            l2 = bb2.instructions
                            if l2 and l2[-1] is ni:
                                l2.pop()
                                removed = True
                                break
                        if removed:
                            break
                    assert removed, "could not relocate wait nop"
                    ni.sync_info = mybir.SyncInfo(on_wait=[w], on_update=[])
                    nops.append(ni)
                inst.sync_info = mybir.SyncInfo(
                    on_wait=[waits[-1]], on_update=list(si.on_update)
                )
                idx = None
                for j in range(len(lst)):
                    if lst[j] is inst:
                        idx = j
                        break
                assert idx is not None
                for k, ni in enumerate(nops):
                    lst.insert(idx + k, ni)
    return nc


def _build_bass():
    nc = bacc.Bacc("TRN2", target_bir_lowering=False)

    # ---- DRAM I/O ----
    x_d = nc.dram_tensor("x", [DIM, NTOK], F32, kind="ExternalInput")
    wqk_d = nc.dram_tensor("wqk", [DIM, 2 * DIM], FP8, kind="ExternalInput")
    wv_d = nc.dram_tensor("wv", [DIM, DIM], FP8, kind="ExternalInput")
    wo_d = nc.dram_tensor("wo", [DIM, DIM], FP8, kind="ExternalInput")
    # corrections: [a_row; b_row] pairs (x WS, fp8)
    cqk_d = nc.dram_tensor("cqk", [1, 2, 2 * DIM], FP8, kind="ExternalInput")
    cv_d = nc.dram_tensor("cv", [1, 2, DIM], FP8, kind="ExternalInput")
    ones2_d = nc.dram_tensor("ones2", [128, 2, 64], FP8, kind="ExternalInput")
    cosb_d = nc.dram_tensor("cosb", [128, WIN], BF16, kind="ExternalInput")
    sinb_d = nc.dram_tensor("sinb", [128, WIN], BF16, kind="ExternalInput")
    st_d = nc.dram_tensor("st128", [128, 128], BF16, kind="ExternalInput")
    e16_d = nc.dram_tensor("e16", [128, HEADS, HEADS], BF16, kind="ExternalInput")
    sel_d = nc.dram_tensor("sel", [HEADS, KC, 128], BF16, kind="ExternalInput")
    out_d = nc.dram_tensor("out", [DIM, NTOK], F32, kind="ExternalOutput")

    x_r = x_d.ap().rearrange("(kc p) n -> p kc n", p=128)
    out_r = out_d.ap().rearrange("(kc p) n -> p kc n", p=128)

    with tile.TileContext(nc) as tc:
        with (
            tc.tile_pool(name="wpool", bufs=1) as wpool,
            tc.tile_pool(name="xpool", bufs=2) as xpool,
            tc.tile_pool(name="spool", bufs=2) as spool,
            tc.tile_pool(name="qkpool", bufs=1) as qkpool,
            tc.tile_pool(name="rpool", bufs=2) as rpool,
            tc.tile_pool(name="tpool", bufs=2) as tpool,
            tc.tile_pool(name="vpool", bufs=2) as vpool,
            tc.tile_pool(name="apool", bufs=2) as apool,
            tc.tile_pool(name="ypool", bufs=2) as ypool,
            tc.tile_pool(name="dpool", bufs=2, space="DRAM") as dpool,
            tc.tile_pool(name="psA", bufs=5, space="PSUM") as psA,
            tc.tile_pool(name="psU", bufs=2, space="PSUM") as psU,
            tc.tile_pool(name="psD", bufs=1, space="PSUM") as psD,
        ):
            # ---- resident weights/constants ----
            wqk = wpool.tile([128, KC, 2 * DIM], FP8)
            nc.sync.dma_start(out=wqk, in_=wqk_d.ap().rearrange("(kc p) m -> p kc m", p=128))
            wv = wpool.tile([128, KC, DIM], FP8)
            nc.sync.dma_start(out=wv, in_=wv_d.ap().rearrange("(kc p) m -> p kc m", p=128))
            wo = wpool.tile([128, KC, DIM], FP8)
            nc.sync.dma_start(out=wo, in_=wo_d.ap().rearrange("(kc p) m -> p kc m", p=128))
            cqk = wpool.tile([1, 2, 2 * DIM], FP8)
            nc.sync.dma_start(out=cqk, in_=cqk_d.ap())
            cv = wpool.tile([1, 2, DIM], FP8)
            nc.sync.dma_start(out=cv, in_=cv_d.ap())
            ones2_t = wpool.tile([128, 2, 64], FP8)
            nc.sync.dma_start(out=ones2_t, in_=ones2_d.ap())
            ones2 = ones2_t[:, :, 0:1]
            cosb = wpool.tile([128, WIN], BF16)
            nc.sync.dma_start(out=cosb, in_=cosb_d.ap())
            sinb = wpool.tile([128, WIN], BF16)
            nc.sync.dma_start(out=sinb, in_=sinb_d.ap())
            st128 = wpool.tile([128, 128], BF16)
            nc.sync.dma_start(out=st128, in_=st_d.ap())
            e16 = wpool.tile([128, HEADS, HEADS], BF16)
            nc.sync.dma_start(out=e16, in_=e16_d.ap())
            sel = wpool.tile([HEADS, KC, 128], BF16)
            nc.sync.dma_start(out=sel, in_=sel_d.ap())
            eps_t = wpool.tile([1, 1], F32)
            nc.vector.memset(eps_t, EPS)
            zero128 = wpool.tile([128, 1], F32)
            nc.vector.memset(zero128, 0.0)

            def bcast_win(ap_2d, nwin):
                """(128, WIN) tile -> (128, nwin, WIN) stride-0 repeat."""
                return bass.AP(
                    tensor=ap_2d.tensor,
                    offset=ap_2d.offset,
                    ap=[ap_2d.ap[0], [0, nwin], ap_2d.ap[1]],
                )

            def bcast_kc(ap_2d, n):
                return bass.AP(
                    tensor=ap_2d.tensor,
                    offset=ap_2d.offset,
                    ap=[ap_2d.ap[0], [0, n], ap_2d.ap[1]],
                )

            def ps3(ps_tile, c, inner):
                """(128, c*inner) psum tile viewed as (128, c, inner)."""
                return bass.AP(
                    tensor=ps_tile.tensor, offset=ps_tile.offset,
                    ap=[ps_tile.ap[0], [inner, c], [1, inner]],
                )

            # ================= phases =================

            def stats_phase(it):
                tb = it * NT
                _mark(nc, f'stats_{it}')
                x8 = xpool.tile([128, KC, NT], F32, tag="x8", bufs=1,
                                name=f"x8_{it}")
                nc.sync.dma_start(out=x8, in_=x_r[:, :, tb : tb + NT])
                xf8 = xpool.tile([128, KC, NT], FP8, tag="xf8", bufs=2,
                                 name=f"xf8_{it}")
                xsq8 = xpool.tile([128, KC, NT], FP8, tag="xsq8", bufs=1,
                                  name=f"xsq8_{it}")
                ps_sum = psA.tile([1, NT], F32, tag="mm1", name=f"ps_sum_{it}")
                ps_sq = psA.tile([1, NT], F32, tag="mm1", name=f"ps_sq_{it}")
                for j in range(KD):
                    ksl = slice(2 * j, 2 * j + 2)
                    nc.gpsimd.tensor_copy(
                        out=xf8[:, ksl, :].rearrange("p k n -> p (k n)"),
                        in_=x8[:, ksl, :].rearrange("p k n -> p (k n)"),
                    )
                    if j % 2 == 0:
                        nc.scalar.square(
                            out=xsq8[:, ksl, :].rearrange("p k n -> p (k n)"),
                            in_=xf8[:, ksl, :].rearrange("p k n -> p (k n)"),
                        )
                    else:
                        nc.gpsimd.tensor_mul(
                            out=xsq8[:, ksl, :].rearrange("p k n -> p (k n)"),
                            in0=xf8[:, ksl, :].rearrange("p k n -> p (k n)"),
                            in1=xf8[:, ksl, :].rearrange("p k n -> p (k n)"),
                        )
                    nc.tensor.matmul(
                        ps_sum[:, :], ones2, xf8[:, ksl, :],
                        start=(j == 0), stop=(j == KD - 1), perf_mode=DR,
                    )
                    nc.tensor.matmul(
                        ps_sq[:, :], ones2, xsq8[:, ksl, :],
                        start=(j == 0), stop=(j == KD - 1), perf_mode=DR,
                    )
                ex = spool.tile([1, NT], F32, tag="sa", bufs=2, name=f"ex_{it}")
                nc.scalar.mul(out=ex, in_=ps_sum[:, :], mul=1.0 / DIM)
                ex2 = spool.tile([1, NT], F32, tag="sb", bufs=1, name=f"ex2_{it}")
                nc.scalar.mul(out=ex2, in_=ps_sq[:, :], mul=1.0 / DIM)
                negex2 = spool.tile([1, NT], F32, tag="sc", bufs=1,
                                    name=f"negex2_{it}")
                nc.vector.scalar_tensor_tensor(
                    out=negex2, in0=ex, scalar=-1.0, in1=ex,
                    op0=mybir.AluOpType.mult, op1=mybir.AluOpType.mult,
                )
                var = spool.tile([1, NT], F32, tag="sa", bufs=2, name=f"var_{it}")
                nc.vector.tensor_add(out=var, in0=ex2, in1=negex2)
                # rstd = exp(-0.5 * ln(var + eps)); keeps ACT in one func set
                lnv = spool.tile([1, NT], F32, tag="sc", bufs=1, name=f"lnv_{it}")
                nc.scalar.activation(
                    out=lnv, in_=var, func=mybir.ActivationFunctionType.Ln,
                    bias=eps_t[:, :], scale=1.0,
                )
                rstd = spool.tile([1, NT], F32, tag="sb", bufs=1,
                                  name=f"rstd_{it}")
                nc.scalar.activation(
                    out=rstd, in_=lnv, func=mybir.ActivationFunctionType.Exp,
                    bias=zero128[0:1, :], scale=-0.5,
                )
                rstd_bf = spool.tile([1, NT], BF16, tag="sbf", bufs=1,
                                     name=f"rstd_bf_{it}")
                nc.gpsimd.tensor_copy(out=rstd_bf, in_=rstd)
                # (1, 2, NT) fp8: row 0 = -mu, row 1 = ones  (rstd is folded
                # into the rope tables / vt evac scale, NOT into the rhs)
                nmr1 = spool.tile([1, 2, NT], FP8, tag="nmr1", bufs=2,
                                  name=f"nmr1_{it}")
                nc.vector.memset(nmr1, 1.0)
                nc.vector.tensor_scalar_mul(out=nmr1[0:1, 0, :], in0=ex,
                                            scalar1=-1.0)
                # broadcast rstd to all partitions; fold into sin/cos tables
                rb = spool.tile([128, NT], BF16, tag="rb", bufs=1,
                                name=f"rb_{it}")
                nc.gpsimd.partition_broadcast(rb, rstd_bf, channels=128)
                srb = spool.tile([128, NT], BF16, tag="srb", bufs=2,
                                 name=f"srb_{it}")
                nc.vector.tensor_mul(out=srb, in0=rb, in1=bcast_win(sinb, WPT))
                crb = spool.tile([128, NT], BF16, tag="crb", bufs=2,
                                 name=f"crb_{it}")
                nc.vector.tensor_mul(out=crb, in0=rb, in1=bcast_win(cosb, WPT))
                # rstd as a per-sub partition column for the vt evac scale
                rsd = dpool.tile([1, NT], F32, tag="rsd", name=f"rsd_{it}")
                nc.sync.dma_start(out=rsd, in_=rstd)
                rstdT = spool.tile([128, WPT], F32, tag="rT", bufs=2,
                                   name=f"rstdT_{it}")
                rsdT_src = bass.AP(tensor=rsd.tensor, offset=rsd.offset,
                                   ap=[[1, 128], [128, WPT]])
                nc.sync.dma_start(out=rstdT, in_=rsdT_src)
                return dict(it=it, tb=tb, xf8=xf8, nmr1=nmr1, srb=srb,
                            crb=crb, rstdT=rstdT)

            def qkrope_phase(st):
                """qk projection fused with rope, 2-mc software offset."""
                it, xf8, nmr1 = st["it"], st["xf8"], st["nmr1"]
                srb, crb = st["srb"], st["crb"]
                _mark(nc, f'qk_{it}')
                qk_e = qkpool.tile([128, 16, NT], BF16, tag="qke", bufs=1,
                                   name=f"qk_e_{it}")
                qs = {}
                roped = rpool.tile([128, 16, NT], BF16, tag="roped", bufs=2,
                                   name=f"roped_{it}")

                def qk_mc(mc):
                    msl = slice(mc * 128, (mc + 1) * 128)
                    ps_qk = psA.tile([128, NT], F32, tag="mm1",
                                     name=f"ps_qk_{it}_{mc}")
                    for j in range(KD):
                        nc.tensor.matmul(
                            ps_qk[:, :],
                            wqk[:, 2 * j : 2 * j + 2, msl],
                            xf8[:, 2 * j : 2 * j + 2, :],
                            start=(j == 0), stop=False, perf_mode=DR,
                        )
                    nc.tensor.matmul(
                        ps_qk[:, :], cqk[:, :, msl], nmr1,
                        start=False, stop=True, perf_mode=DR,
                    )
                    nc.scalar.copy(out=qk_e[:, mc, :], in_=ps_qk[:, :])
                    qs[mc] = tpool.tile([128, NT], BF16, tag="qs", bufs=4,
                                        name=f"qs_{it}_{mc}")
                    nc.vector.tensor_mul(
                        out=qs[mc], in0=qk_e[:, mc, :], in1=srb,
                    )

                def rope_mc(mc):
                    ps_u = psU.tile([128, NT], F32, tag="uu",
                                    name=f"ps_u_{it}_{mc}")
                    nc.tensor.matmul(ps_u[:, :], st128, qs.pop(mc),
                                     start=True, stop=True)
                    qc = tpool.tile([128, NT], BF16, tag="qc", bufs=2,
                                    name=f"qc_{it}_{mc}")
                    nc.gpsimd.tensor_mul(
                        out=qc, in0=qk_e[:, mc, :], in1=crb)
                    nc.vector.tensor_add(
                        out=roped[:, mc, :], in0=ps_u[:, :], in1=qc)

                for mc in range(16):
                    qk_mc(mc)
                    if mc >= 2:
                        rope_mc(mc - 2)
                rope_mc(14)
                rope_mc(15)
                st["roped"] = roped

            def v_phase(st):
                it, xf8, nmr1 = st["it"], st["xf8"], st["nmr1"]
                rstdT = st["rstdT"]
                _mark(nc, f'v_{it}')
                vt = vpool.tile([128, WPT, DIM], BF16, tag="vt", bufs=2,
                                name=f"vt_{it}")
                for sub in range(WPT):
                    ssl = slice(sub * 128, (sub + 1) * 128)
                    for nh in range(2):
                        ncol = slice(nh * 512, (nh + 1) * 512)
                        ps_vt = psA.tile([128, 512], F32, tag="mm1",
                                         name=f"ps_vt_{it}_{sub}_{nh}")
                        for j in range(KD):
                            nc.tensor.matmul(
                                ps_vt[:, :],
                                xf8[:, 2 * j : 2 * j + 2, ssl],
                                wv[:, 2 * j : 2 * j + 2, ncol],
                                start=(j == 0), stop=False, perf_mode=DR,
                            )
                        nc.tensor.matmul(
                            ps_vt[:, :], nmr1[:, :, ssl], cv[:, :, ncol],
                            start=False, stop=True, perf_mode=DR,
                        )
                        nc.scalar.activation(
                            out=vt[:, sub, ncol], in_=ps_vt[:, :],
                            func=mybir.ActivationFunctionType.Copy,
                            bias=0.0, scale=rstdT[:, sub : sub + 1],
                        )
                st["vt"] = vt

            def ecol(hh):
                return (hh % 2) * 512 + (hh // 2) * WIN

            def attn_scores(st, wl):
                it, roped = st["it"], st["roped"]
                _mark(nc, f'attn_{it}_{wl}')
                wslc = slice(wl * WIN, (wl + 1) * WIN)
                expt = apool.tile([128, 2, 8 * WIN], BF16, tag="expt", bufs=2,
                                  name=f"expt_{it}_{wl}")
                # parity-split: matmuls with different operand base partition
                # (0 vs 64) never share a PSUM bank.
                for hg in range(2):
                    for par in range(2):
                        ps_sc = psA.tile([128, 4 * WIN], F32, tag="mm1",
                                         name=f"ps_sc_{it}_{wl}_{hg}_{par}")
                        po = par * 64
                        for j in range(4):
                            h = hg * 8 + 2 * j + par
                            qh = roped[po : po + 64, h // 2, wslc]
                            kh = roped[po : po + 64, 8 + h // 2, wslc]
                            nc.tensor.matmul(
                                ps_sc[:, j * WIN : (j + 1) * WIN],
                                kh, qh, start=True, stop=True,
                            )
                        nc.scalar.activation(
                            out=expt[:, hg, par * 512 : (par + 1) * 512],
                            in_=ps_sc[:, :],
                            func=mybir.ActivationFunctionType.Exp,
                            bias=zero128[:, :], scale=0.125 / (WS * WS),
                        )
                st[f"expt_{wl}"] = expt

            def attn_tail(st, wl):
                it, vt = st["it"], st["vt"]
                expt = st.pop(f"expt_{wl}")
                attn_t = st["attn_t"]
                wslc = slice(wl * WIN, (wl + 1) * WIN)
                ps_d = psD.tile([HEADS, WIN], F32, tag="dd",
                                name=f"ps_d_{it}_{wl}")
                for hg in range(2):
                    for hh in range(8):
                        h = hg * 8 + hh
                        nc.tensor.matmul(
                            ps_d[:, :], e16[:, h, :],
                            expt[:, hg, ecol(hh) : ecol(hh) + WIN],
                            start=(h == 0), stop=(h == HEADS - 1),
                        )
                rd = spool.tile([HEADS, WIN], F32, tag="rd", bufs=2,
                                name=f"rd_{it}_{wl}")
                nc.vector.reciprocal_approx_fast(out=rd, in_=ps_d[:, :])
                rd_bf = spool.tile([HEADS, WIN], BF16, tag="rd", bufs=2,
                                   name=f"rd_bf_{it}_{wl}")
                nc.gpsimd.tensor_copy(out=rd_bf, in_=rd)
                # broadcast rd to (128, kc, WIN) layout via sel matmuls
                bcb = apool.tile([128, KC, WIN], BF16, tag="bcb", bufs=2,
                                 name=f"bcb_{it}_{wl}")
                for hg in range(2):
                    ps_bc = psA.tile([128, 4 * WIN], F32, tag="mm1",
                                     name=f"ps_bc_{it}_{wl}_{hg}")
                    for cc in range(4):
                        c = hg * 4 + cc
                        nc.tensor.matmul(
                            ps_bc[:, cc * WIN : (cc + 1) * WIN],
                            sel[:, c, :], rd_bf, start=True, stop=True,
                        )
                    nc.scalar.copy(
                        out=bcb[:, 4 * hg : 4 * hg + 4, :],
                        in_=ps3(ps_bc, 4, WIN),
                    )
                for hg in range(2):
                    ps_at = psA.tile([128, 4 * WIN], F32, tag="mm1",
                                     name=f"ps_at_{it}_{wl}_{hg}")
                    for hh in range(8):
                        h = hg * 8 + hh
                        po = (h % 2) * 64
                        c = (h // 2) % 4
                        nc.tensor.matmul(
                            ps_at[po : po + 64, c * WIN : (c + 1) * WIN],
                            vt[:, wl, h * 64 : (h + 1) * 64],
                            expt[:, hg, ecol(hh) : ecol(hh) + WIN],
                            start=True, stop=True,
                            tile_position=(0, po),
                        )
                    # fused evac + normalize: psum * bcast -> fp8 attn_t
                    nc.vector.tensor_mul(
                        out=attn_t[:, 4 * hg : 4 * hg + 4, wslc],
                        in0=ps3(ps_at, 4, WIN),
                        in1=bcb[:, 4 * hg : 4 * hg + 4, :],
                    )

            def outproj_phase(st):
                it, tb, attn_t = st["it"], st["tb"], st["attn_t"]
                _mark(nc, f'outproj_{it}')
                for mc in range(KC):
                    msl = slice(mc * 128, (mc + 1) * 128)
                    ps_y = psA.tile([128, NT], F32, tag="mm1",
                                    name=f"ps_y_{it}_{mc}")
                    for j in range(KD):
                        nc.tensor.matmul(
                            ps_y[:, :],
                            wo[:, 2 * j : 2 * j + 2, msl],
                            attn_t[:, 2 * j : 2 * j + 2, :],
                            start=(j == 0), stop=(j == KD - 1), perf_mode=DR,
                        )
                    xres = ypool.tile([128, NT], F32, tag="xres", bufs=2,
                                      name=f"xres_{it}_{mc}")
                    nc.sync.dma_start(out=xres, in_=x_r[:, mc, tb : tb + NT])
                    y = ypool.tile([128, NT], F32, tag="y", bufs=4,
                                   name=f"y_{it}_{mc}")
                    nc.vector.scalar_tensor_tensor(
                        out=y, in0=ps_y[:, :], scalar=1.0 / (WS * WS),
                        in1=xres,
                        op0=mybir.AluOpType.mult, op1=mybir.AluOpType.add,
                    )
                    nc.sync.dma_start(
                        out=out_r[:, mc, tb : tb + NT], in_=y,
                    )

            def attn_all(st):
                attn_scores(st, 0)
                attn_scores(st, 1)
                attn_tail(st, 0)
                attn_scores(st, 2)
                attn_tail(st, 1)
                attn_scores(st, 3)
                attn_tail(st, 2)
                attn_tail(st, 3)

            # ============ software pipeline ============
            # per iter i: stats(i) | attn(i-1) windows pipelined | qk+rope(i)
            #             | outproj(i-1) | v(i)
            prev = None
            cur = stats_phase(0)
            for it in range(NTILES):
                cur["attn_t"] = apool.tile([128, KC, NT], FP8, tag="attn",
                                           bufs=2, name=f"attn_t_{it}")
                nxt = stats_phase(it + 1) if it + 1 < NTILES else None
                if prev is not None:
                    attn_all(prev)
                qkrope_phase(cur)
                if prev is not None:
                    outproj_phase(prev)
                v_phase(cur)
                prev, cur = cur, nxt
            attn_all(prev)
            outproj_phase(prev)

    nc.finalize()
    return nc


def _host_prep(x, ln_w, ln_b, w_qkv, w_out):
    """Shared (non-x) device inputs, host-precomputed."""
    f8 = ml_dtypes.float8_e4m3fn if hasattr(ml_dtypes, 'float8_e4m3fn') \
        else ml_dtypes.float8_e4m3
    bf = ml_dtypes.bfloat16
    wqkv_s = (w_qkv * ln_w[None, :]).astype(np.float32)  # (3C, C) scaled
    wT = np.ascontiguousarray(wqkv_s.T)  # (C, 3C)
    b_qkv = (w_qkv @ ln_b).astype(np.float32)  # (3C,)
    a_qkv = wqkv_s.sum(axis=1).astype(np.float32)  # (3C,)

    ins = {}
    ins["wqk"] = np.ascontiguousarray(wT[:, : 2 * DIM] * WS).astype(f8)
    ins["wv"] = np.ascontiguousarray(wT[:, 2 * DIM :] * WS).astype(f8)
    ins["wo"] = np.ascontiguousarray(w_out.T * WS).astype(f8)
    ins["cqk"] = np.stack(
        [a_qkv[: 2 * DIM] * WS, b_qkv[: 2 * DIM] * WS]
    )[None].astype(f8)
    ins["cv"] = np.stack(
        [a_qkv[2 * DIM :] * WS, b_qkv[2 * DIM :] * WS]
    )[None].astype(f8)
    ins["ones2"] = np.ones((128, 2, 64), np.float32).astype(f8)

    inv_freq = 1.0 / 10000 ** (np.arange(0, DHEAD, 2, dtype=np.float32) / DHEAD)
    pos = np.arange(WIN, dtype=np.float32)
    freqs = np.concatenate([np.outer(pos, inv_freq)] * 2, axis=-1)  # (WIN, 64)
    cos_t = np.cos(freqs).T.astype(np.float32)  # (64, WIN)
    sin_t = np.sin(freqs).T.astype(np.float32)
    ins["cosb"] = np.tile(cos_t, (2, 1)).astype(bf)
    ins["sinb"] = np.tile(sin_t, (2, 1)).astype(bf)

    S = np.zeros((DHEAD, DHEAD), np.float32)
    S[: DHEAD // 2, DHEAD // 2 :] = -np.eye(DHEAD // 2)
    S[DHEAD // 2 :, : DHEAD // 2] = np.eye(DHEAD // 2)
    ST = S.T
    st128 = np.zeros((128, 128), np.float32)
    st128[:64, :64] = ST
    st128[64:, 64:] = ST
    ins["st128"] = st128.astype(bf)

    e = np.zeros((128, HEADS, HEADS), np.float32)
    for h in range(HEADS):
        e[:, h, h] = 1.0
    ins["e16"] = e.astype(bf)

    # sel[h, c, p] = 1 iff head(p, c) == h, i.e. h == 2c + (p >= 64)
    s = np.zeros((HEADS, KC, 128), np.float32)
    for c in range(KC):
        s[2 * c, c, :64] = 1.0
        s[2 * c + 1, c, 64:] = 1.0
    ins["sel"] = s.astype(bf)
    return ins


def kernel(x, ln_w, ln_b, w_qkv, w_out, _want_trace=False):
    x = np.asarray(x, dtype=np.float32)
    shared = _host_prep(
        np.asarray(x, np.float32),
        np.asarray(ln_w, np.float32),
        np.asarray(ln_b, np.float32),
        np.asarray(w_qkv, np.float32),
        np.asarray(w_out, np.float32),
    )

    if "nc" not in _CACHED:
        _CACHED["nc"] = _build_bass()
    nc = _CACHED["nc"]

    in_maps = []
    for core in range(NCORE):
        b, half = core // 2, core % 2
        xs = np.ascontiguousarray(x[b, :, half * NTOK : (half + 1) * NTOK])
        m = dict(shared)
        m["x"] = xs
        in_maps.append(m)

    res = run_bass_kernel_spmd(
        nc, in_maps, core_ids=list(range(NCORE)), trace=_want_trace
    )
    out = np.empty((B, DIM, T), np.float32)
    for core in range(NCORE):
        b, half = core // 2, core % 2
        out[b, :, half * NTOK : (half + 1) * NTOK] = res.results[core]["out"]
    if _want_trace:
        _CACHED["last_trace"] = res
    return out
